# revision 1
# baseline (speedup 1.0000x reference)
"""Trainium2 Bass kernel for nn_ClassConditionalDriftingLoss.

Math per class c (G gen rows, P pos rows, D=64, T=G+P targets):
  d2[t,g]  = ||x_t||^2 + ||y_g||^2 - 2 x_t.y_g          (x=targets, y=gen)
  k        = exp(-2.5*sqrt(d2)),  k[diag]=0             (dist normalized by sqrt(D)=8,
                                                         TEMP=0.05 -> exp(-20*dist/8*... ) = exp(-2.5*sqrt(d2)))
  row[g]   = sum_t k[g,t];  col[t] = sum_g k[g,t]
  nk       = k * min(rsqrt(row[g])*rsqrt(col[t]), 1e6)  (== k / sqrt(max(row*col,1e-12)))
  s_gen[g] = sum_{t<G} nk ; s_pos[g] = sum_{t>=G} nk
  M_pos    = nk[:,G:] @ pos ; M_neg = nk[:,:G] @ gen
  V        = s_gen*M_pos - s_pos*M_neg
  loss    += sum(V^2);  drift += sum_g ||V[g]||

The kernel matrix is held transposed (Kt[t,g], t on partitions) in SBUF bf16.
One class per NeuronCore (8 classes / 8 cores), no collectives; host sums the
8 scalar pairs.

exp/ln only (single ACT table set `natural_log_exp_and_others`):
  sqrt(z) = exp(0.5*ln(z)); rsqrt(z) = exp(-0.5*ln(z)).
Diagonal masking: a BIG*I matmul accumulated onto the d2 PSUM drives those
entries to exp(-sqrt(6.25e9)) == 0.
"""

import sys

for _p in ("/opt/trn_rl_repo", "/root/.axon_site/_ro/trn_rl_repo"):
    if _p not in sys.path:
        sys.path.insert(0, _p)

import math

import ml_dtypes
import numpy as np

C = 8
BIG = 1.0e9  # added to diagonal d2 entries -> k underflows to exactly 0
EPS_LN = 0.01  # ln(6.25*d2 + EPS_LN); guards d2 ~ -1e-4 roundoff on the diagonal

_CACHE = {}


def _patch_act_tables():
    """Keep every ACT func only in natural_log_exp_and_others so the
    table-load inserter never thrashes between sets (Ln/Exp/Copy all live
    there; set IDs stay aligned with the compiler's act_info.json)."""
    import functools

    import concourse.bacc as bacc
    import concourse.hw_specs as hw_specs

    if getattr(hw_specs.get_activation_tables, "_drift_patched", False):
        return
    orig = hw_specs.get_activation_tables

    @functools.cache
    def patched(module_arch):
        keep = "natural_log_exp_and_others"
        return {
            name: (funcs if name == keep else set())
            for name, funcs in orig(module_arch).items()
        }

    patched._drift_patched = True
    hw_specs.get_activation_tables = patched
    bacc.get_activation_tables = patched


def _build(G, P):
    import concourse.bacc as bacc
    import concourse.tile as tile
    from concourse import mybir

    _patch_act_tables()

    f32 = mybir.dt.float32
    bf16 = mybir.dt.bfloat16
    AF = mybir.ActivationFunctionType
    OP = mybir.AluOpType

    T = G + P
    NT = T // 128  # t-chunks (partition dim of Kt)
    NG = G // 128  # gen-block t-chunks
    GS = G // 512  # 512-wide g slices
    RW = 512  # matmul slice width (psum bank limit: 512 fp32)
    GR = G // RW
    assert T % 128 == 0 and G % 512 == 0 and P % 128 == 0

    nc = bacc.Bacc("TRN2", target_bir_lowering=False, debug=False, num_devices=8)

    ta = nc.dram_tensor("ta", [66, T], f32, kind="ExternalInput")
    gb = nc.dram_tensor("gb", [66, G], f32, kind="ExternalInput")
    posa = nc.dram_tensor("posa", [128, P // 128, 65], bf16, kind="ExternalInput")
    gena = nc.dram_tensor("gena", [128, G // 128, 65], bf16, kind="ExternalInput")
    ident = nc.dram_tensor("ident", [128, 128], bf16, kind="ExternalInput")
    identf = nc.dram_tensor("identf", [128, 128], f32, kind="ExternalInput")
    bigi = nc.dram_tensor("bigi", [128, 128], bf16, kind="ExternalInput")
    ones_r = nc.dram_tensor("ones_r", [128, 1], bf16, kind="ExternalInput")
    ones_b = nc.dram_tensor("ones_b", [1, 128], f32, kind="ExternalInput")
    ones_v = nc.dram_tensor("ones_v", [64, 1], f32, kind="ExternalInput")
    outh = nc.dram_tensor("out", [1, 2], f32, kind="ExternalOutput")

    from contextlib import ExitStack

    with tile.TileContext(nc) as tc, ExitStack() as ctx:
        kpool = ctx.enter_context(tc.tile_pool(name="kpool", bufs=1))
        bigp = ctx.enter_context(tc.tile_pool(name="bigp", bufs=2))
        singles = ctx.enter_context(tc.tile_pool(name="singles", bufs=1))
        spool = ctx.enter_context(tc.tile_pool(name="spool", bufs=1))
        smalls = ctx.enter_context(tc.tile_pool(name="smalls", bufs=1))
        wpool = ctx.enter_context(tc.tile_pool(name="wpool", bufs=2))
        vtp = ctx.enter_context(tc.tile_pool(name="vtp", bufs=1))

        # persistent kernel matrix, [128, NT, G] bf16 (t-chunk major)
        KT = kpool.tile([128, NT, G], bf16)

        TAs = bigp.tile([66, T], f32, tag="b16", bufs=1)
        GBs = bigp.tile([66, G], f32, tag="b8", bufs=2)
        # DMA order matters: the supertile loop starts at the pos block
        # (i=NG), so its TA quarter + the first GB half + ones_r must land
        # first; late-needed tensors ride the queue tails.
        # keep the ACT queue DMA-free: its first instruction must be the
        # activation-table load, not a bulk transfer
        h, q3 = T // 2, 3 * T // 4
        nc.sync.dma_start(out=TAs[:, h:q3], in_=ta[:, h:q3])
        nc.sync.dma_start(out=TAs[:, q3:], in_=ta[:, q3:])
        nc.gpsimd.dma_start(out=GBs[:, : G // 2], in_=gb[:, : G // 2])
        nc.gpsimd.dma_start(out=GBs[:, G // 2 :], in_=gb[:, G // 2 :])

        POSAs = singles.tile([128, P // 128, 65], bf16)
        GENAs = singles.tile([128, G // 128, 65], bf16)
        IDENTs = singles.tile([128, 128], bf16)
        IDENTFs = singles.tile([128, 128], f32)
        BIGIs = singles.tile([128, 128], bf16)
        ONESRs = singles.tile([128, 1], bf16)
        ONESBs = singles.tile([1, 128], f32)
        ONESVs = singles.tile([64, 1], f32)
        nc.gpsimd.dma_start(out=ONESRs[:], in_=ones_r[:, :])
        nc.gpsimd.dma_start(out=POSAs[:], in_=posa[:, :, :])
        nc.gpsimd.dma_start(out=GENAs[:], in_=gena[:, :, :])
        nc.sync.dma_start(out=IDENTs[:], in_=ident[:, :])
        nc.sync.dma_start(out=IDENTFs[:], in_=identf[:, :])
        nc.sync.dma_start(out=BIGIs[:], in_=bigi[:, :])
        nc.sync.dma_start(out=ONESBs[:], in_=ones_b[:, :])
        nc.sync.dma_start(out=ONESVs[:], in_=ones_v[:, :])
        nc.sync.dma_start(out=TAs[:, :h], in_=ta[:, :h])

        colacc = smalls.tile([128, NT], f32)
        lnc = smalls.tile([128, NT], f32)
        bvec = smalls.tile([128, NT], f32)
        epsb = smalls.tile([128, 1], f32)
        qeps = smalls.tile([1, 1], f32)
        nc.vector.memset(epsb[:], EPS_LN)
        nc.vector.memset(qeps[:], 1.0e-35)

        # ---- Phase 1: build Kt, col sums (ACT accum), row sums (PE ones-matmul)
        with (
            tc.tile_pool(name="d2p", bufs=1, space="PSUM") as dp,
            tc.tile_pool(name="rap", bufs=1, space="PSUM") as rp,
        ):
            rowaccs = [
                rp.tile([1, RW], f32, tag=f"ra{j}", name=f"rowacc{j}")
                for j in range(GR)
            ]
            # pos chunks first so each rowacc bank opens with a full-width
            # start=True matmul; gen-chunk transposes then accumulate slices
            for i in list(range(NG, NT)) + list(range(NG)):
                d2 = dp.tile([128, G], f32, tag="d2")
                jd = (i * 128) // 512 if i < NG else -1  # g-slice holding the diagonal
                for j in range(GS):
                    nc.tensor.matmul(
                        d2[:, j * 512 : (j + 1) * 512],
                        TAs[:, i * 128 : (i + 1) * 128],
                        GBs[:, j * 512 : (j + 1) * 512],
                        start=True,
                        stop=(j != jd),
                        skip_group_check=True,
                    )
                if jd >= 0:
                    nc.tensor.matmul(
                        d2[:, i * 128 : i * 128 + 128],
                        IDENTs[:],
                        BIGIs[:],
                        start=False,
                        stop=True,
                        skip_group_check=True,
                    )
                S = spool.tile([128, G], f32, tag="scratch")
                # u = ln(6.25*d2 + eps); v = exp(0.5*u) = 2.5*sqrt(d2+eps');
                # k = exp(-v) -> bf16, accumulating column sums per partition
                nc.scalar.activation(S[:], d2[:], AF.Ln, bias=epsb[:], scale=6.25)
                nc.scalar.activation(S[:], S[:], AF.Exp, scale=0.5)
                nc.scalar.activation(
                    KT[:, i, :], S[:], AF.Exp, scale=-1.0,
                    accum_out=colacc[:, i : i + 1],
                )
                # row sums: pos block via ones-matmul; gen block comes free
                # from colacc (gen-gen block of Kt is symmetric:
                # row_gen[128i+p] == colacc[p, i]), transposed into the same
                # accumulators by a colacc[:, i] x identity matmul.
                if i < NG:
                    j, m = divmod(i * 128, RW)
                    nc.tensor.matmul(
                        rowaccs[j][:, m : m + 128],
                        colacc[:, i : i + 1],
                        IDENTFs[:],
                        start=False,
                        stop=(m + 128 == RW),
                        skip_group_check=True,
                    )
                else:
                    for j in range(GR):
                        nc.tensor.matmul(
                            rowaccs[j][:],
                            ONESRs[:],
                            KT[:, i, j * RW : (j + 1) * RW],
                            start=(i == NG),
                            stop=False,
                            skip_group_check=True,
                        )
            # row sums -> a = rsqrt(row) (in place, single-partition vector)
            # a = rsqrt(row), per slice straight from PSUM (no copy chain)
            rowS = spool.tile([1, G], f32, tag="scratch")
            for j in range(GR):
                js = slice(j * RW, (j + 1) * RW)
                nc.scalar.activation(rowS[:, js], rowaccs[j][:], AF.Ln)
                nc.scalar.activation(rowS[:, js], rowS[:, js], AF.Exp, scale=-0.5)

        # b = rsqrt(col) per t (per-partition, [128, NT])
        nc.scalar.activation(lnc[:], colacc[:], AF.Ln)
        nc.scalar.activation(bvec[:], lnc[:], AF.Exp, scale=-0.5)

        # broadcast a across partitions -> [128, G] bf16
        BCA = singles.tile([128, G], bf16)
        with tc.tile_pool(name="bcp", bufs=2, space="PSUM") as bp:
            for j in range(GS):
                pb = bp.tile([128, 512], f32, tag="bc")
                nc.tensor.matmul(
                    pb[:], ONESBs[:], rowS[:, j * 512 : (j + 1) * 512],
                    start=True, stop=True, skip_group_check=True,
                )
                nc.vector.tensor_copy(BCA[:, j * 512 : (j + 1) * 512], pb[:])

        # ---- Phase 1.75 + 2a: nk = k*min(a*b, 1e6) (in place), side matmuls
        with tc.tile_pool(name="p2p", bufs=1, space="PSUM") as p2:
            psums = {}
            for side in range(2):
                for j in range(GR):
                    psums[(side, j)] = p2.tile(
                        [65, RW], f32, tag=f"p2_{side}_{j}", name=f"p2_{side}_{j}"
                    )
            for i in range(NT):
                W = wpool.tile([128, G], bf16, tag="w")
                nc.vector.tensor_scalar(
                    W[:], BCA[:], bvec[:, i : i + 1], 1.0e6, op0=OP.mult, op1=OP.min
                )
                nc.vector.tensor_mul(KT[:, i, :], KT[:, i, :], W[:])
                side = 0 if i < NG else 1
                lhs = GENAs[:, i, :] if side == 0 else POSAs[:, i - NG, :]
                first = i in (0, NG)
                last = i in (NG - 1, NT - 1)
                for j in range(GR):
                    nc.tensor.matmul(
                        psums[(side, j)][:],
                        lhs,
                        KT[:, i, j * RW : (j + 1) * RW],
                        start=first,
                        stop=last,
                        skip_group_check=True,
                    )
            # rows 0:64 = M_neg.T / M_pos.T ; row 64 = s_gen / s_pos
            PNo = bigp.tile([65, G], f32, tag="b8", bufs=2)
            PPo = bigp.tile([65, G], f32, tag="b8", bufs=2)
            for j in range(GR):
                js = slice(j * RW, (j + 1) * RW)
                nc.vector.tensor_copy(PNo[:, js], psums[(0, j)][:])
                nc.vector.tensor_copy(PPo[:, js], psums[(1, j)][:])

        # ---- Phase 2b: V.T = bcast(s_gen)*M_pos.T - bcast(s_pos)*M_neg.T
        qS = spool.tile([1, G], f32, tag="scratch")
        lsums = smalls.tile([1, GS], f32)
        dsums = smalls.tile([1, GS], f32)
        with (
            tc.tile_pool(name="bc2", bufs=2, space="PSUM") as bp2,
            tc.tile_pool(name="qp", bufs=2, space="PSUM") as qp,
        ):
            for j in range(GS):
                js = slice(j * 512, (j + 1) * 512)
                sgr = vtp.tile([1, 512], f32, tag="sgr")
                spr = vtp.tile([1, 512], f32, tag="spr")
                nc.vector.tensor_copy(sgr[:], PNo[64:65, js])
                nc.vector.tensor_copy(spr[:], PPo[64:65, js])
                bg = bp2.tile([64, 512], f32, tag="bg")
                bpp = bp2.tile([64, 512], f32, tag="bp")
                nc.tensor.matmul(
                    bg[:], ONESBs[:, 0:64], sgr[:],
                    start=True, stop=True, skip_group_check=True,
                )
                nc.tensor.matmul(
                    bpp[:], ONESBs[:, 0:64], spr[:],
                    start=True, stop=True, skip_group_check=True,
                )
                vt1 = vtp.tile([64, 512], f32, tag="vt1", bufs=2)
                vt2 = vtp.tile([64, 512], f32, tag="vt2", bufs=2)
                nc.vector.tensor_mul(vt1[:], PPo[0:64, js], bg[:])
                nc.vector.tensor_mul(vt2[:], PNo[0:64, js], bpp[:])
                nc.vector.tensor_sub(vt1[:], vt1[:], vt2[:])
                nc.vector.tensor_mul(vt2[:], vt1[:], vt1[:])  # V^2
                qt = qp.tile([1, 512], f32, tag="q")
                nc.tensor.matmul(
                    qt[:], ONESVs[:], vt2[:], start=True, stop=True,
                    skip_group_check=True,
                )
                # loss partial = sum(q) via copy-with-accum; drift partial =
                # sum(sqrt(q)) via ln/exp with accum - all inside the loop so
                # nothing serializes at the end of the kernel
                nc.vector.tensor_scalar(
                    qS[:, js], qt[:], 1.0, 0.0, op0=OP.mult, op1=OP.add,
                    accum_out=lsums[:, j : j + 1],
                )
                nc.scalar.activation(qS[:, js], qS[:, js], AF.Ln, bias=qeps[:])
                nc.scalar.activation(
                    qS[:, js], qS[:, js], AF.Exp, scale=0.5,
                    accum_out=dsums[:, j : j + 1],
                )

        outS = smalls.tile([1, 2], f32)
        nc.vector.tensor_reduce(
            outS[:, 0:1], lsums[:], axis=mybir.AxisListType.X, op=OP.add
        )
        nc.vector.tensor_reduce(
            outS[:, 1:2], dsums[:], axis=mybir.AxisListType.X, op=OP.add
        )
        nc.sync.dma_start(out=outh[:, :], in_=outS[:])

    nc.compile()
    return nc


def _prep_class(gen_c, pos_c):
    """Host-side input prep for one class -> dict of named arrays."""
    gen_c = np.ascontiguousarray(gen_c, dtype=np.float32)
    pos_c = np.ascontiguousarray(pos_c, dtype=np.float32)
    G, D = gen_c.shape
    P = pos_c.shape[0]
    T = G + P
    targets = np.concatenate([gen_c, pos_c], axis=0)

    ta = np.empty((66, T), np.float32)
    ta[0:64] = -2.0 * targets.T
    ta[64] = (targets * targets).sum(axis=1)
    ta[65] = 1.0

    gbm = np.empty((66, G), np.float32)
    gbm[0:64] = gen_c.T
    gbm[64] = 1.0
    gbm[65] = (gen_c * gen_c).sum(axis=1)

    def aug(x):
        n = x.shape[0]
        a = np.empty((n, 65), np.float32)
        a[:, 0:64] = x
        a[:, 64] = 1.0
        return (
            a.astype(ml_dtypes.bfloat16)
            .reshape(n // 128, 128, 65)
            .transpose(1, 0, 2)
            .copy()
        )

    bf = ml_dtypes.bfloat16
    return {
        "ta": ta,
        "gb": gbm,
        "posa": aug(pos_c),
        "gena": aug(gen_c),
        "ident": np.eye(128, dtype=bf),
        "identf": np.eye(128, dtype=np.float32),
        "bigi": (BIG * np.eye(128)).astype(bf),
        "ones_r": np.ones((128, 1), bf),
        "ones_b": np.ones((1, 128), np.float32),
        "ones_v": np.ones((64, 1), np.float32),
    }


def kernel(generated, labels_gen, positive, labels_pos):
    from concourse.bass_utils import run_bass_kernel_spmd

    generated = np.asarray(generated, dtype=np.float32)
    positive = np.asarray(positive, dtype=np.float32)
    N, D = generated.shape
    Np = positive.shape[0]
    G, P = N // C, Np // C
    assert D == 64

    key = (G, P)
    if key not in _CACHE:
        _CACHE[key] = _build(G, P)
    nc = _CACHE[key]

    in_maps = [
        _prep_class(
            generated[c * G : (c + 1) * G], positive[c * P : (c + 1) * P]
        )
        for c in range(C)
    ]
    res = run_bass_kernel_spmd(nc, in_maps, core_ids=list(range(C)))
    sums = np.stack([res.results[i]["out"][0] for i in range(C)])  # [C, 2]
    loss = sums[:, 0].sum() / (N * D)
    dn = sums[:, 1].sum() / N
    return np.float32(loss), np.float32(dn)


if __name__ == "__main__":
    rng = np.random.default_rng(0)
    N = 16384
    gen = rng.standard_normal((N, 64), dtype=np.float32)
    pos = rng.standard_normal((N, 64), dtype=np.float32)
    lg = np.repeat(np.arange(C), N // C).astype(np.int32)
    print(kernel(gen, lg, pos, lg))



# revision 20
# speedup vs baseline: 1.1200x; 1.1200x over previous
"""Trainium2 Bass kernel for nn_ClassConditionalDriftingLoss.

Math per class c (G gen rows, P pos rows, D=64, T=G+P targets):
  d2[t,g]  = ||x_t||^2 + ||y_g||^2 - 2 x_t.y_g          (x=targets, y=gen)
  k        = exp(-2.5*sqrt(d2)),  k[diag]=0             (dist normalized by sqrt(D)=8,
                                                         TEMP=0.05 -> exp(-2.5*sqrt(d2)))
  row[g]   = sum_t k[t,g];  col[t] = sum_g k[t,g]
  nk       = k * min(rsqrt(row)*rsqrt(col), 1e6)        (== k / sqrt(max(row*col,1e-12)))
  s_gen[g] = sum_{t<G} nk ; s_pos[g] = sum_{t>=G} nk
  M_pos    = nk[G:].T @ pos ; M_neg = nk[:G].T @ gen    (transposed layout)
  V        = s_gen*M_pos - s_pos*M_neg
  loss    += sum(V^2);  drift += sum_g ||V[g]||

One class per NeuronCore (8 classes / 8 cores), no collectives; host sums
the 8 scalar pairs.

Implementation notes:
  - d2 via Gram matmuls in float32r (fp32 data, 1 cycle/row on PE for
    512-wide outputs vs 4 for plain fp32).
  - 2-pass ACT chain instead of the 3-pass ln/exp/exp: S = sqrt(6.25*d2
    + eps) then k = exp(-S) with column-sum accumulation.  sqrt and exp
    live in different activation-table sets, so chunks are processed in
    groups of GK between table switches (1283ns per load), with S parked
    in a [128, GK, G] f32 scratch.
  - Row sums: pos chunks via ones-matmuls; the gen-gen block is
    symmetric so its row sums equal colacc (transposed into the same
    PSUM accumulators by a colacc x identity matmul).
  - Phase 2 re-reads the persistent bf16 KT: W = min(bcast(rsqrt(row)) *
    rsqrt(col)_t, 1e6) (DVE 4x mode), nk chunk = KT*W (DVE 2x), side
    matmuls psum[side] += aug(side).T @ nk with aug = [x, 1] so row 64
    holds s_gen/s_pos.
  - Diagonal masking: BIG*I matmul accumulated onto the d2 PSUM makes
    k_diag = exp(-sqrt(6.25*BIG)) underflow to 0 in bf16.
"""

import sys

for _p in ("/opt/trn_rl_repo", "/root/.axon_site/_ro/trn_rl_repo"):
    if _p not in sys.path:
        sys.path.insert(0, _p)

import ml_dtypes
import numpy as np

C = 8
BIG = 4000.0  # diag d2 offset: exp(-sqrt(6.25*4000)) = exp(-158) -> 0 in bf16
EPS = 0.01  # sqrt(6.25*d2 + EPS); guards d2 ~ -1e-4 roundoff on the diagonal
GK = 4  # chunks per activation-table group

_CACHE = {}


def _build(G, P):
    import concourse.bacc as bacc
    import concourse.tile as tile
    from concourse import mybir

    f32 = mybir.dt.float32
    f32r = mybir.dt.float32r
    bf16 = mybir.dt.bfloat16
    AF = mybir.ActivationFunctionType
    OP = mybir.AluOpType

    T = G + P
    NT = T // 128  # t-chunks (partition dim of KT)
    NG = G // 128  # gen-block t-chunks
    RW = 512  # matmul slice width (psum bank limit: 512 fp32)
    GR = G // RW
    HW = G // 2  # d2 half width (2 psum banks per half)
    assert T % 128 == 0 and G % 512 == 0 and P % 128 == 0 and NT % GK == 0

    nc = bacc.Bacc("TRN2", target_bir_lowering=False, debug=False, num_devices=8)

    ta = nc.dram_tensor("ta", [66, T], f32r, kind="ExternalInput")
    gb = nc.dram_tensor("gb", [66, G], f32r, kind="ExternalInput")
    posa = nc.dram_tensor("posa", [128, P // 128, 65], bf16, kind="ExternalInput")
    gena = nc.dram_tensor("gena", [128, G // 128, 65], bf16, kind="ExternalInput")
    ident = nc.dram_tensor("ident", [128, 128], bf16, kind="ExternalInput")
    identf = nc.dram_tensor("identf", [128, 128], f32, kind="ExternalInput")
    bigi = nc.dram_tensor("bigi", [128, 128], bf16, kind="ExternalInput")
    ones_r = nc.dram_tensor("ones_r", [128, 1], bf16, kind="ExternalInput")
    ones_b = nc.dram_tensor("ones_b", [1, 128], f32, kind="ExternalInput")
    ones_v = nc.dram_tensor("ones_v", [64, 1], f32, kind="ExternalInput")
    outh = nc.dram_tensor("out", [1, 2], f32, kind="ExternalOutput")

    from contextlib import ExitStack

    with tile.TileContext(nc) as tc, ExitStack() as ctx:
        kpool = ctx.enter_context(tc.tile_pool(name="kpool", bufs=1))
        bigp = ctx.enter_context(tc.tile_pool(name="bigp", bufs=1))
        singles = ctx.enter_context(tc.tile_pool(name="singles", bufs=1))
        smalls = ctx.enter_context(tc.tile_pool(name="smalls", bufs=1))

        # persistent kernel matrix, [128, NT, G] bf16 (t-chunk major)
        KT = kpool.tile([128, NT, G], bf16)

        POSAs = singles.tile([128, P // 128, 65], bf16)
        GENAs = singles.tile([128, G // 128, 65], bf16)
        IDENTs = singles.tile([128, 128], bf16)
        IDENTFs = singles.tile([128, 128], f32)
        BIGIs = singles.tile([128, 128], bf16)
        ONESRs = singles.tile([128, 1], bf16)
        ONESBs = singles.tile([1, 128], f32)
        ONESVs = singles.tile([64, 1], f32)

        epsb = smalls.tile([128, 1], f32)
        qeps = smalls.tile([1, 1], f32)
        colacc = smalls.tile([128, NT], f32)
        lnc = smalls.tile([128, NT], f32)
        bvec = smalls.tile([128, NT], f32)
        lsums = smalls.tile([1, GR], f32)
        dsums = smalls.tile([1, GR], f32)

        rowS = bigp.tile([1, G], f32, tag="rowS")
        BCA = bigp.tile([128, G], bf16, tag="bca")

        order = list(range(NG, NT)) + list(range(NG))  # pos chunks first

        # phase-1-only tensors live in their own pool, freed before phase 2
        with tc.tile_pool(name="ph1", bufs=1) as p1:
            TAs = p1.tile([66, T], f32r, tag="ta")
            GBs = p1.tile([66, G], f32r, tag="gb")
            S = p1.tile([128, GK, G], f32, tag="s")  # sqrt scratch

            # DMA order: the chunk loop starts at the pos block (i=NG), so
            # its TA quarter + GB must land first; late-needed tensors ride
            # the queue tails.  ACT queue stays DMA-free (first instruction
            # must be the sqrt table load).
            h, q3 = T // 2, 3 * T // 4
            nc.sync.dma_start(out=TAs[:, h:q3], in_=ta[:, h:q3])
            nc.sync.dma_start(out=TAs[:, q3:], in_=ta[:, q3:])
            nc.gpsimd.dma_start(out=GBs[:, : G // 2], in_=gb[:, : G // 2])
            nc.gpsimd.dma_start(out=GBs[:, G // 2 :], in_=gb[:, G // 2 :])
            nc.gpsimd.dma_start(out=ONESRs[:], in_=ones_r[:, :])
            nc.gpsimd.dma_start(out=POSAs[:], in_=posa[:, :, :])
            nc.gpsimd.dma_start(out=GENAs[:], in_=gena[:, :, :])
            nc.sync.dma_start(out=IDENTs[:], in_=ident[:, :])
            nc.sync.dma_start(out=IDENTFs[:], in_=identf[:, :])
            nc.sync.dma_start(out=BIGIs[:], in_=bigi[:, :])
            nc.sync.dma_start(out=ONESBs[:], in_=ones_b[:, :])
            nc.sync.dma_start(out=ONESVs[:], in_=ones_v[:, :])
            nc.sync.dma_start(out=TAs[:, :h], in_=ta[:, :h])

            nc.vector.memset(epsb[:], EPS)
            nc.vector.memset(qeps[:], 1.0e-35)

            groups = [order[g * GK : (g + 1) * GK] for g in range(NT // GK)]

            # ---- Phase 1: build KT (sqrt/exp table groups), col + row sums
            with (
                tc.tile_pool(name="d2p", bufs=1, space="PSUM") as dp,
                tc.tile_pool(name="rap", bufs=1, space="PSUM") as rp,
            ):
                rowaccs = [
                    rp.tile([1, RW], f32, tag=f"ra{j}", name=f"rowacc{j}")
                    for j in range(GR)
                ]
                for grp in groups:
                    # sqrt phase: d2 matmuls ping-pong 2 psum half-tiles
                    for slot, i in enumerate(grp):
                        jd = i // (RW // 128) if i < NG else -1
                        for hh in range(2):
                            d2 = dp.tile(
                                [128, HW], f32, tag=f"d2{hh}", name=f"d2{hh}"
                            )
                            for jj in range(HW // RW):
                                j = hh * (HW // RW) + jj
                                nc.tensor.matmul(
                                    d2[:, jj * RW : (jj + 1) * RW],
                                    TAs[:, i * 128 : (i + 1) * 128],
                                    GBs[:, j * RW : (j + 1) * RW],
                                    start=True,
                                    stop=(j != jd),
                                    skip_group_check=True,
                                )
                                if j == jd:
                                    c0 = i * 128 - hh * HW
                                    nc.tensor.matmul(
                                        d2[:, c0 : c0 + 128],
                                        IDENTs[:],
                                        BIGIs[:],
                                        start=False,
                                        stop=True,
                                        skip_group_check=True,
                                    )
                            nc.scalar.activation(
                                S[:, slot, hh * HW : (hh + 1) * HW],
                                d2[:],
                                AF.Sqrt,
                                bias=epsb[:],
                                scale=6.25,
                            )
                    # exp phase: k chunks -> KT + col sums; row-sum matmuls
                    for slot, i in enumerate(grp):
                        nc.scalar.activation(
                            KT[:, i, :], S[:, slot, :], AF.Exp, scale=-1.0,
                            accum_out=colacc[:, i : i + 1],
                        )
                        if i < NG:
                            # gen-gen block symmetric: its row sums ==
                            # colacc; transpose into the rowacc banks
                            j, m = divmod(i * 128, RW)
                            nc.tensor.matmul(
                                rowaccs[j][:, m : m + 128],
                                colacc[:, i : i + 1],
                                IDENTFs[:],
                                start=False,
                                stop=(m + 128 == RW),
                                skip_group_check=True,
                            )
                        else:
                            for j in range(GR):
                                nc.tensor.matmul(
                                    rowaccs[j][:],
                                    ONESRs[:],
                                    KT[:, i, j * RW : (j + 1) * RW],
                                    start=(i == NG),
                                    stop=False,
                                    skip_group_check=True,
                                )
                # a = rsqrt(row) straight from PSUM (exp/ln table is loaded)
                for j in range(GR):
                    js = slice(j * RW, (j + 1) * RW)
                    nc.scalar.activation(rowS[:, js], rowaccs[j][:], AF.Ln)
                    nc.scalar.activation(
                        rowS[:, js], rowS[:, js], AF.Exp, scale=-0.5
                    )

            # b = rsqrt(col) per t (per-partition, [128, NT])
            nc.scalar.activation(lnc[:], colacc[:], AF.Ln)
            nc.scalar.activation(bvec[:], lnc[:], AF.Exp, scale=-0.5)

            # broadcast a across partitions -> [128, G] bf16
            with tc.tile_pool(name="bcp", bufs=2, space="PSUM") as bp:
                for j in range(GR):
                    pb = bp.tile([128, RW], f32, tag="bc")
                    nc.tensor.matmul(
                        pb[:], ONESBs[:], rowS[:, j * RW : (j + 1) * RW],
                        start=True, stop=True, skip_group_check=True,
                    )
                    nc.vector.tensor_copy(BCA[:, j * RW : (j + 1) * RW], pb[:])

        # ---- Phase 2: nk chunks = KT * min(a*b, 1e6), side matmuls
        # (ph2 pool opens after ph1 closed, reusing its SBUF region)
        ph2 = ctx.enter_context(tc.tile_pool(name="ph2", bufs=1))
        PNo = ph2.tile([65, G], f32, tag="pn")
        PPo = ph2.tile([65, G], f32, tag="pp")
        with (
            tc.tile_pool(name="wp", bufs=2) as wp,
            tc.tile_pool(name="nkp", bufs=3) as nkp,
            tc.tile_pool(name="sp2", bufs=1, space="PSUM") as p2,
        ):
            psums = None
            for i in order:
                if i in (NG, 0):  # entering pos / gen half: (re)use banks
                    psums = [
                        p2.tile([65, RW], f32, tag=f"sp{j}", name=f"sp{j}_{i}")
                        for j in range(GR)
                    ]
                W = wp.tile([128, G], bf16, tag="w")
                nc.vector.tensor_scalar(
                    W[:], BCA[:], bvec[:, i : i + 1], 1.0e6,
                    op0=OP.mult, op1=OP.min,
                )
                NK = nkp.tile([128, G], bf16, tag="nk")
                nc.vector.tensor_mul(NK[:], KT[:, i, :], W[:])
                side = 0 if i < NG else 1
                lhs = GENAs[:, i, :] if side == 0 else POSAs[:, i - NG, :]
                first = i in (0, NG)
                last = i in (NG - 1, NT - 1)
                for j in range(GR):
                    nc.tensor.matmul(
                        psums[j][:],
                        lhs,
                        NK[:, j * RW : (j + 1) * RW],
                        start=first,
                        stop=last,
                        skip_group_check=True,
                    )
                if i == NT - 1:  # pos side complete -> drain
                    for j in range(GR):
                        nc.vector.tensor_copy(
                            PPo[:, j * RW : (j + 1) * RW], psums[j][:]
                        )
                if i == NG - 1:  # gen side complete -> drain
                    for j in range(GR):
                        nc.vector.tensor_copy(
                            PNo[:, j * RW : (j + 1) * RW], psums[j][:]
                        )

        # ---- tail: V.T = bcast(s_gen)*M_pos.T - bcast(s_pos)*M_neg.T
        # rows 0:64 of PNo/PPo = M_neg.T / M_pos.T ; row 64 = s_gen / s_pos
        qS = bigp.tile([1, G], f32, tag="rowS")  # reuse the rowS region
        with (
            tc.tile_pool(name="vtp", bufs=1) as vtp,
            tc.tile_pool(name="bc2", bufs=2, space="PSUM") as bp2,
            tc.tile_pool(name="qp", bufs=2, space="PSUM") as qp,
        ):
            for j in range(GR):
                js = slice(j * RW, (j + 1) * RW)
                sgr = vtp.tile([1, RW], f32, tag="sgr")
                spr = vtp.tile([1, RW], f32, tag="spr")
                nc.vector.tensor_copy(sgr[:], PNo[64:65, js])
                nc.vector.tensor_copy(spr[:], PPo[64:65, js])
                bg = bp2.tile([64, RW], f32, tag="bg")
                bpp = bp2.tile([64, RW], f32, tag="bp")
                nc.tensor.matmul(
                    bg[:], ONESBs[:, 0:64], sgr[:],
                    start=True, stop=True, skip_group_check=True,
                )
                nc.tensor.matmul(
                    bpp[:], ONESBs[:, 0:64], spr[:],
                    start=True, stop=True, skip_group_check=True,
                )
                vt1 = vtp.tile([64, RW], f32, tag="vt1", bufs=2)
                vt2 = vtp.tile([64, RW], f32, tag="vt2", bufs=2)
                nc.vector.tensor_mul(vt1[:], PPo[0:64, js], bg[:])
                nc.vector.tensor_mul(vt2[:], PNo[0:64, js], bpp[:])
                nc.vector.tensor_sub(vt1[:], vt1[:], vt2[:])
                nc.vector.tensor_mul(vt2[:], vt1[:], vt1[:])  # V^2
                qt = qp.tile([1, RW], f32, tag="q")
                nc.tensor.matmul(
                    qt[:], ONESVs[:], vt2[:], start=True, stop=True,
                    skip_group_check=True,
                )
                # loss partial = sum(q) via copy-with-accum; drift partial =
                # sum(sqrt(q)) via ln/exp with accum
                nc.vector.tensor_scalar(
                    qS[:, js], qt[:], 1.0, 0.0, op0=OP.mult, op1=OP.add,
                    accum_out=lsums[:, j : j + 1],
                )
                nc.scalar.activation(qS[:, js], qS[:, js], AF.Ln, bias=qeps[:])
                nc.scalar.activation(
                    qS[:, js], qS[:, js], AF.Exp, scale=0.5,
                    accum_out=dsums[:, j : j + 1],
                )

        outS = smalls.tile([1, 2], f32)
        nc.vector.tensor_reduce(
            outS[:, 0:1], lsums[:], axis=mybir.AxisListType.X, op=OP.add
        )
        nc.vector.tensor_reduce(
            outS[:, 1:2], dsums[:], axis=mybir.AxisListType.X, op=OP.add
        )
        nc.sync.dma_start(out=outh[:, :], in_=outS[:])

    nc.compile()
    return nc


def _prep_class(gen_c, pos_c):
    """Host-side input prep for one class -> dict of named arrays."""
    gen_c = np.ascontiguousarray(gen_c, dtype=np.float32)
    pos_c = np.ascontiguousarray(pos_c, dtype=np.float32)
    G, D = gen_c.shape
    targets = np.concatenate([gen_c, pos_c], axis=0)
    T = targets.shape[0]

    ta = np.empty((66, T), np.float32)
    ta[0:64] = -2.0 * targets.T
    ta[64] = (targets * targets).sum(axis=1)
    ta[65] = 1.0

    gbm = np.empty((66, G), np.float32)
    gbm[0:64] = gen_c.T
    gbm[64] = 1.0
    gbm[65] = (gen_c * gen_c).sum(axis=1)

    def aug(x):
        n = x.shape[0]
        a = np.empty((n, 65), np.float32)
        a[:, 0:64] = x
        a[:, 64] = 1.0
        return (
            a.astype(ml_dtypes.bfloat16)
            .reshape(n // 128, 128, 65)
            .transpose(1, 0, 2)
            .copy()
        )

    bf = ml_dtypes.bfloat16
    return {
        "ta": ta,
        "gb": gbm,
        "posa": aug(pos_c),
        "gena": aug(gen_c),
        "ident": np.eye(128, dtype=bf),
        "identf": np.eye(128, dtype=np.float32),
        "bigi": (BIG * np.eye(128)).astype(bf),
        "ones_r": np.ones((128, 1), bf),
        "ones_b": np.ones((1, 128), np.float32),
        "ones_v": np.ones((64, 1), np.float32),
    }


def kernel(generated, labels_gen, positive, labels_pos):
    from concourse.bass_utils import run_bass_kernel_spmd

    generated = np.asarray(generated, dtype=np.float32)
    positive = np.asarray(positive, dtype=np.float32)
    N, D = generated.shape
    Np = positive.shape[0]
    G, P = N // C, Np // C
    assert D == 64

    key = (G, P)
    if key not in _CACHE:
        _CACHE[key] = _build(G, P)
    nc = _CACHE[key]

    in_maps = [
        _prep_class(
            generated[c * G : (c + 1) * G], positive[c * P : (c + 1) * P]
        )
        for c in range(C)
    ]
    res = run_bass_kernel_spmd(nc, in_maps, core_ids=list(range(C)))
    sums = np.stack([res.results[i]["out"][0] for i in range(C)])  # [C, 2]
    loss = sums[:, 0].sum() / (N * D)
    dn = sums[:, 1].sum() / N
    return np.float32(loss), np.float32(dn)


if __name__ == "__main__":
    rng = np.random.default_rng(0)
    N = 16384
    gen = rng.standard_normal((N, 64), dtype=np.float32)
    pos = rng.standard_normal((N, 64), dtype=np.float32)
    lg = np.repeat(np.arange(C), N // C).astype(np.int32)
    print(kernel(gen, lg, pos, lg))


# revision 23
# speedup vs baseline: 1.2183x; 1.0878x over previous
"""Trainium2 Bass kernel for nn_ClassConditionalDriftingLoss.

Math per class c (G gen rows, P pos rows, D=64, T=G+P targets):
  d2[t,g]  = ||x_t||^2 + ||y_g||^2 - 2 x_t.y_g          (x=targets, y=gen)
  k        = exp(-2.5*sqrt(d2)),  k[diag]=0             (dist normalized by sqrt(D)=8,
                                                         TEMP=0.05 -> exp(-2.5*sqrt(d2)))
  row[g]   = sum_t k[t,g];  col[t] = sum_g k[t,g]
  nk       = k * min(rsqrt(row)*rsqrt(col), 1e6)        (== k / sqrt(max(row*col,1e-12)))
  s_gen[g] = sum_{t<G} nk ; s_pos[g] = sum_{t>=G} nk
  M_pos    = nk[G:].T @ pos ; M_neg = nk[:G].T @ gen    (transposed layout)
  V        = s_gen*M_pos - s_pos*M_neg
  loss    += sum(V^2);  drift += sum_g ||V[g]||

One class per NeuronCore (8 classes / 8 cores), no collectives; host sums
the 8 scalar pairs.

Implementation notes:
  - d2 via Gram matmuls in float32r (fp32 data, 1 cycle/row on PE for
    512-wide outputs vs 4 for plain fp32).
  - 2-pass ACT chain instead of the 3-pass ln/exp/exp: S = sqrt(6.25*d2
    + eps) then k = exp(-S) with column-sum accumulation.  sqrt and exp
    live in different activation-table sets, so chunks are processed in
    groups of GK between table switches (1283ns per load), with S parked
    in a [128, GK, G] f32 scratch.
  - Row sums: pos chunks via ones-matmuls; the gen-gen block is
    symmetric so its row sums equal colacc (transposed into the same
    PSUM accumulators by a colacc x identity matmul).
  - Phase 2 re-reads the persistent bf16 KT: W = min(bcast(rsqrt(row)) *
    rsqrt(col)_t, 1e6) (DVE 4x mode), nk chunk = KT*W (DVE 2x), side
    matmuls psum[side] += aug(side).T @ nk with aug = [x, 1] so row 64
    holds s_gen/s_pos.
  - Diagonal masking: BIG*I matmul accumulated onto the d2 PSUM makes
    k_diag = exp(-sqrt(6.25*BIG)) underflow to 0 in bf16.
"""

import sys

for _p in ("/opt/trn_rl_repo", "/root/.axon_site/_ro/trn_rl_repo"):
    if _p not in sys.path:
        sys.path.insert(0, _p)

import ml_dtypes
import numpy as np

C = 8
BIG = 4000.0  # diag d2 offset: exp(-sqrt(6.25*4000)) = exp(-158) -> 0 in bf16
EPS = 0.01  # sqrt(6.25*d2 + EPS); guards d2 ~ -1e-4 roundoff on the diagonal
GK = 4  # chunks per activation-table group

_CACHE = {}


def _patch_act_tables():
    """Restrict ACT table sets so the load inserter never thrashes: Sqrt
    lives only in sqrt_and_others, Ln/Exp only in natural_log_exp_and_others
    (the default sets split ln and exp across natural_log / exp_and_others,
    which makes every Ln->Exp pair reload a table).  Set IDs stay aligned
    with the compiler's act_info.json because only membership is filtered."""
    import functools

    import concourse.bacc as bacc
    import concourse.hw_specs as hw_specs

    if getattr(hw_specs.get_activation_tables, "_drift_patched", False):
        return
    orig = hw_specs.get_activation_tables

    @functools.cache
    def patched(module_arch):
        keep = ("natural_log_exp_and_others", "sqrt_and_others")
        return {
            name: (funcs if name in keep else set())
            for name, funcs in orig(module_arch).items()
        }

    patched._drift_patched = True
    hw_specs.get_activation_tables = patched
    bacc.get_activation_tables = patched


def _build(G, P):
    import concourse.bacc as bacc
    import concourse.tile as tile
    from concourse import mybir

    _patch_act_tables()

    f32 = mybir.dt.float32
    f32r = mybir.dt.float32r
    bf16 = mybir.dt.bfloat16
    AF = mybir.ActivationFunctionType
    OP = mybir.AluOpType

    T = G + P
    NT = T // 128  # t-chunks (partition dim of KT)
    NG = G // 128  # gen-block t-chunks
    RW = 512  # matmul slice width (psum bank limit: 512 fp32)
    GR = G // RW
    HW = G // 2  # d2 half width (2 psum banks per half)
    assert T % 128 == 0 and G % 512 == 0 and P % 128 == 0 and NT % GK == 0

    nc = bacc.Bacc("TRN2", target_bir_lowering=False, debug=False, num_devices=8)

    ta = nc.dram_tensor("ta", [66, T], f32r, kind="ExternalInput")
    gb = nc.dram_tensor("gb", [66, G], f32r, kind="ExternalInput")
    posa = nc.dram_tensor("posa", [128, P // 128, 65], bf16, kind="ExternalInput")
    gena = nc.dram_tensor("gena", [128, G // 128, 65], bf16, kind="ExternalInput")
    ident = nc.dram_tensor("ident", [128, 128], bf16, kind="ExternalInput")
    identf = nc.dram_tensor("identf", [128, 128], f32, kind="ExternalInput")
    bigi = nc.dram_tensor("bigi", [128, 128], bf16, kind="ExternalInput")
    ones_r = nc.dram_tensor("ones_r", [128, 1], bf16, kind="ExternalInput")
    ones_b = nc.dram_tensor("ones_b", [1, 128], f32, kind="ExternalInput")
    ones_v = nc.dram_tensor("ones_v", [64, 1], f32, kind="ExternalInput")
    outh = nc.dram_tensor("out", [1, 2], f32, kind="ExternalOutput")

    from contextlib import ExitStack

    with tile.TileContext(nc) as tc, ExitStack() as ctx:
        kpool = ctx.enter_context(tc.tile_pool(name="kpool", bufs=1))
        bigp = ctx.enter_context(tc.tile_pool(name="bigp", bufs=1))
        singles = ctx.enter_context(tc.tile_pool(name="singles", bufs=1))
        smalls = ctx.enter_context(tc.tile_pool(name="smalls", bufs=1))

        # persistent kernel matrix, [128, NT, G] bf16 (t-chunk major)
        KT = kpool.tile([128, NT, G], bf16)

        POSAs = singles.tile([128, P // 128, 65], bf16)
        GENAs = singles.tile([128, G // 128, 65], bf16)
        IDENTs = singles.tile([128, 128], bf16)
        IDENTFs = singles.tile([128, 128], f32)
        BIGIs = singles.tile([128, 128], bf16)
        ONESRs = singles.tile([128, 1], bf16)
        ONESBs = singles.tile([1, 128], f32)
        ONESVs = singles.tile([64, 1], f32)

        epsb = smalls.tile([128, 1], f32)
        qeps = smalls.tile([1, 1], f32)
        colacc = smalls.tile([128, NT], f32)
        lnc = smalls.tile([128, NT], f32)
        bvec = smalls.tile([128, NT], f32)
        lsums = smalls.tile([1, GR], f32)
        dsums = smalls.tile([1, GR], f32)

        rowS = bigp.tile([1, G], f32, tag="rowS")
        BCA = bigp.tile([128, G], bf16, tag="bca")

        order = list(range(NG, NT)) + list(range(NG))  # pos chunks first

        # phase-1-only tensors live in their own pool, freed before phase 2
        with tc.tile_pool(name="ph1", bufs=1) as p1:
            TAs = p1.tile([66, T], f32r, tag="ta")
            GBs = p1.tile([66, G], f32r, tag="gb")
            S = p1.tile([128, GK, G], f32, tag="s")  # sqrt scratch

            # DMA order: the chunk loop starts at the pos block (i=NG), so
            # its TA quarter + GB must land first; late-needed tensors ride
            # the queue tails.  ACT queue stays DMA-free (first instruction
            # must be the sqrt table load).
            h, q3 = T // 2, 3 * T // 4
            nc.sync.dma_start(out=TAs[:, h:q3], in_=ta[:, h:q3])
            nc.sync.dma_start(out=TAs[:, q3:], in_=ta[:, q3:])
            nc.gpsimd.dma_start(out=GBs[:, : G // 2], in_=gb[:, : G // 2])
            nc.gpsimd.dma_start(out=GBs[:, G // 2 :], in_=gb[:, G // 2 :])
            nc.gpsimd.dma_start(out=ONESRs[:], in_=ones_r[:, :])
            nc.gpsimd.dma_start(out=POSAs[:], in_=posa[:, :, :])
            nc.gpsimd.dma_start(out=GENAs[:], in_=gena[:, :, :])
            nc.sync.dma_start(out=IDENTs[:], in_=ident[:, :])
            nc.sync.dma_start(out=IDENTFs[:], in_=identf[:, :])
            nc.sync.dma_start(out=BIGIs[:], in_=bigi[:, :])
            nc.sync.dma_start(out=ONESBs[:], in_=ones_b[:, :])
            nc.sync.dma_start(out=ONESVs[:], in_=ones_v[:, :])
            nc.sync.dma_start(out=TAs[:, :h], in_=ta[:, :h])

            nc.vector.memset(epsb[:], EPS)
            nc.vector.memset(qeps[:], 1.0e-35)

            groups = [order[g * GK : (g + 1) * GK] for g in range(NT // GK)]

            # ---- Phase 1: build KT (sqrt/exp table groups), col + row sums
            with (
                tc.tile_pool(name="d2p", bufs=1, space="PSUM") as dp,
                tc.tile_pool(name="rap", bufs=1, space="PSUM") as rp,
            ):
                rowaccs = [
                    rp.tile([1, RW], f32, tag=f"ra{j}", name=f"rowacc{j}")
                    for j in range(GR)
                ]
                for grp in groups:
                    # sqrt phase: d2 matmuls ping-pong 2 psum half-tiles
                    for slot, i in enumerate(grp):
                        jd = i // (RW // 128) if i < NG else -1
                        for hh in range(2):
                            d2 = dp.tile(
                                [128, HW], f32, tag=f"d2{hh}", name=f"d2{hh}"
                            )
                            for jj in range(HW // RW):
                                j = hh * (HW // RW) + jj
                                nc.tensor.matmul(
                                    d2[:, jj * RW : (jj + 1) * RW],
                                    TAs[:, i * 128 : (i + 1) * 128],
                                    GBs[:, j * RW : (j + 1) * RW],
                                    start=True,
                                    stop=(j != jd),
                                    skip_group_check=True,
                                )
                                if j == jd:
                                    c0 = i * 128 - hh * HW
                                    nc.tensor.matmul(
                                        d2[:, c0 : c0 + 128],
                                        IDENTs[:],
                                        BIGIs[:],
                                        start=False,
                                        stop=True,
                                        skip_group_check=True,
                                    )
                            nc.scalar.activation(
                                S[:, slot, hh * HW : (hh + 1) * HW],
                                d2[:],
                                AF.Sqrt,
                                bias=epsb[:],
                                scale=6.25,
                            )
                    # exp phase: k chunks -> KT + col sums; row-sum matmuls
                    for slot, i in enumerate(grp):
                        nc.scalar.activation(
                            KT[:, i, :], S[:, slot, :], AF.Exp, scale=-1.0,
                            accum_out=colacc[:, i : i + 1],
                        )
                        if i < NG:
                            # gen-gen block symmetric: its row sums ==
                            # colacc; transpose into the rowacc banks
                            j, m = divmod(i * 128, RW)
                            nc.tensor.matmul(
                                rowaccs[j][:, m : m + 128],
                                colacc[:, i : i + 1],
                                IDENTFs[:],
                                start=False,
                                stop=(m + 128 == RW),
                                skip_group_check=True,
                            )
                        else:
                            for j in range(GR):
                                nc.tensor.matmul(
                                    rowaccs[j][:],
                                    ONESRs[:],
                                    KT[:, i, j * RW : (j + 1) * RW],
                                    start=(i == NG),
                                    stop=False,
                                    skip_group_check=True,
                                )
                # a = rsqrt(row) straight from PSUM (exp/ln table is loaded)
                for j in range(GR):
                    js = slice(j * RW, (j + 1) * RW)
                    nc.scalar.activation(rowS[:, js], rowaccs[j][:], AF.Ln)
                    nc.scalar.activation(
                        rowS[:, js], rowS[:, js], AF.Exp, scale=-0.5
                    )

            # b = rsqrt(col) per t (per-partition, [128, NT])
            nc.scalar.activation(lnc[:], colacc[:], AF.Ln)
            nc.scalar.activation(bvec[:], lnc[:], AF.Exp, scale=-0.5)

            # broadcast a across partitions -> [128, G] bf16
            with tc.tile_pool(name="bcp", bufs=2, space="PSUM") as bp:
                for j in range(GR):
                    pb = bp.tile([128, RW], f32, tag="bc")
                    nc.tensor.matmul(
                        pb[:], ONESBs[:], rowS[:, j * RW : (j + 1) * RW],
                        start=True, stop=True, skip_group_check=True,
                    )
                    nc.vector.tensor_copy(BCA[:, j * RW : (j + 1) * RW], pb[:])

        # ---- Phase 2: nk chunks = KT * min(a*b, 1e6), side matmuls
        # (ph2 pool opens after ph1 closed, reusing its SBUF region)
        ph2 = ctx.enter_context(tc.tile_pool(name="ph2", bufs=1))
        PNo = ph2.tile([65, G], f32, tag="pn")
        PPo = ph2.tile([65, G], f32, tag="pp")
        with (
            tc.tile_pool(name="wp", bufs=2) as wp,
            tc.tile_pool(name="nkp", bufs=3) as nkp,
            tc.tile_pool(name="sp2", bufs=1, space="PSUM") as p2,
        ):
            psums = None
            for i in order:
                if i in (NG, 0):  # entering pos / gen half: (re)use banks
                    psums = [
                        p2.tile([65, RW], f32, tag=f"sp{j}", name=f"sp{j}_{i}")
                        for j in range(GR)
                    ]
                W = wp.tile([128, G], bf16, tag="w")
                nc.vector.tensor_scalar(
                    W[:], BCA[:], bvec[:, i : i + 1], 1.0e6,
                    op0=OP.mult, op1=OP.min,
                )
                NK = nkp.tile([128, G], bf16, tag="nk")
                # DVE is the phase-2 bottleneck: offload every 3rd chunk's
                # nk multiply to the otherwise-idle GpSimd engine
                eng = nc.gpsimd if i % 3 == 2 else nc.vector
                eng.tensor_mul(NK[:], KT[:, i, :], W[:])
                side = 0 if i < NG else 1
                lhs = GENAs[:, i, :] if side == 0 else POSAs[:, i - NG, :]
                first = i in (0, NG)
                last = i in (NG - 1, NT - 1)
                for j in range(GR):
                    nc.tensor.matmul(
                        psums[j][:],
                        lhs,
                        NK[:, j * RW : (j + 1) * RW],
                        start=first,
                        stop=last,
                        skip_group_check=True,
                    )
                if i == NT - 1:  # pos side complete -> drain
                    for j in range(GR):
                        nc.vector.tensor_copy(
                            PPo[:, j * RW : (j + 1) * RW], psums[j][:]
                        )
                if i == NG - 1:  # gen side complete -> drain
                    for j in range(GR):
                        nc.vector.tensor_copy(
                            PNo[:, j * RW : (j + 1) * RW], psums[j][:]
                        )

        # ---- tail: V.T = bcast(s_gen)*M_pos.T - bcast(s_pos)*M_neg.T
        # rows 0:64 of PNo/PPo = M_neg.T / M_pos.T ; row 64 = s_gen / s_pos
        qS = bigp.tile([1, G], f32, tag="rowS")  # reuse the rowS region
        with (
            tc.tile_pool(name="vtp", bufs=1) as vtp,
            tc.tile_pool(name="bc2", bufs=2, space="PSUM") as bp2,
            tc.tile_pool(name="qp", bufs=2, space="PSUM") as qp,
        ):
            for j in range(GR):
                js = slice(j * RW, (j + 1) * RW)
                sgr = vtp.tile([1, RW], f32, tag="sgr")
                spr = vtp.tile([1, RW], f32, tag="spr")
                nc.vector.tensor_copy(sgr[:], PNo[64:65, js])
                nc.vector.tensor_copy(spr[:], PPo[64:65, js])
                bg = bp2.tile([64, RW], f32, tag="bg")
                bpp = bp2.tile([64, RW], f32, tag="bp")
                nc.tensor.matmul(
                    bg[:], ONESBs[:, 0:64], sgr[:],
                    start=True, stop=True, skip_group_check=True,
                )
                nc.tensor.matmul(
                    bpp[:], ONESBs[:, 0:64], spr[:],
                    start=True, stop=True, skip_group_check=True,
                )
                vt1 = vtp.tile([64, RW], f32, tag="vt1", bufs=2)
                vt2 = vtp.tile([64, RW], f32, tag="vt2", bufs=2)
                nc.vector.tensor_mul(vt1[:], PPo[0:64, js], bg[:])
                nc.vector.tensor_mul(vt2[:], PNo[0:64, js], bpp[:])
                nc.vector.tensor_sub(vt1[:], vt1[:], vt2[:])
                nc.vector.tensor_mul(vt2[:], vt1[:], vt1[:])  # V^2
                qt = qp.tile([1, RW], f32, tag="q")
                nc.tensor.matmul(
                    qt[:], ONESVs[:], vt2[:], start=True, stop=True,
                    skip_group_check=True,
                )
                # loss partial = sum(q) via copy-with-accum; drift partial =
                # sum(sqrt(q)) via ln/exp with accum
                nc.vector.tensor_scalar(
                    qS[:, js], qt[:], 1.0, 0.0, op0=OP.mult, op1=OP.add,
                    accum_out=lsums[:, j : j + 1],
                )
                nc.scalar.activation(qS[:, js], qS[:, js], AF.Ln, bias=qeps[:])
                nc.scalar.activation(
                    qS[:, js], qS[:, js], AF.Exp, scale=0.5,
                    accum_out=dsums[:, j : j + 1],
                )

        outS = smalls.tile([1, 2], f32)
        nc.vector.tensor_reduce(
            outS[:, 0:1], lsums[:], axis=mybir.AxisListType.X, op=OP.add
        )
        nc.vector.tensor_reduce(
            outS[:, 1:2], dsums[:], axis=mybir.AxisListType.X, op=OP.add
        )
        nc.sync.dma_start(out=outh[:, :], in_=outS[:])

    nc.compile()
    return nc


def _prep_class(gen_c, pos_c):
    """Host-side input prep for one class -> dict of named arrays."""
    gen_c = np.ascontiguousarray(gen_c, dtype=np.float32)
    pos_c = np.ascontiguousarray(pos_c, dtype=np.float32)
    G, D = gen_c.shape
    targets = np.concatenate([gen_c, pos_c], axis=0)
    T = targets.shape[0]

    ta = np.empty((66, T), np.float32)
    ta[0:64] = -2.0 * targets.T
    ta[64] = (targets * targets).sum(axis=1)
    ta[65] = 1.0

    gbm = np.empty((66, G), np.float32)
    gbm[0:64] = gen_c.T
    gbm[64] = 1.0
    gbm[65] = (gen_c * gen_c).sum(axis=1)

    def aug(x):
        n = x.shape[0]
        a = np.empty((n, 65), np.float32)
        a[:, 0:64] = x
        a[:, 64] = 1.0
        return (
            a.astype(ml_dtypes.bfloat16)
            .reshape(n // 128, 128, 65)
            .transpose(1, 0, 2)
            .copy()
        )

    bf = ml_dtypes.bfloat16
    return {
        "ta": ta,
        "gb": gbm,
        "posa": aug(pos_c),
        "gena": aug(gen_c),
        "ident": np.eye(128, dtype=bf),
        "identf": np.eye(128, dtype=np.float32),
        "bigi": (BIG * np.eye(128)).astype(bf),
        "ones_r": np.ones((128, 1), bf),
        "ones_b": np.ones((1, 128), np.float32),
        "ones_v": np.ones((64, 1), np.float32),
    }


def kernel(generated, labels_gen, positive, labels_pos):
    from concourse.bass_utils import run_bass_kernel_spmd

    generated = np.asarray(generated, dtype=np.float32)
    positive = np.asarray(positive, dtype=np.float32)
    N, D = generated.shape
    Np = positive.shape[0]
    G, P = N // C, Np // C
    assert D == 64

    key = (G, P)
    if key not in _CACHE:
        _CACHE[key] = _build(G, P)
    nc = _CACHE[key]

    in_maps = [
        _prep_class(
            generated[c * G : (c + 1) * G], positive[c * P : (c + 1) * P]
        )
        for c in range(C)
    ]
    res = run_bass_kernel_spmd(nc, in_maps, core_ids=list(range(C)))
    sums = np.stack([res.results[i]["out"][0] for i in range(C)])  # [C, 2]
    loss = sums[:, 0].sum() / (N * D)
    dn = sums[:, 1].sum() / N
    return np.float32(loss), np.float32(dn)


if __name__ == "__main__":
    rng = np.random.default_rng(0)
    N = 16384
    gen = rng.standard_normal((N, 64), dtype=np.float32)
    pos = rng.standard_normal((N, 64), dtype=np.float32)
    lg = np.repeat(np.arange(C), N // C).astype(np.int32)
    print(kernel(gen, lg, pos, lg))


# revision 25
# speedup vs baseline: 1.2561x; 1.0311x over previous
"""Trainium2 Bass kernel for nn_ClassConditionalDriftingLoss.

Math per class c (G gen rows, P pos rows, D=64, T=G+P targets):
  d2[t,g]  = ||x_t||^2 + ||y_g||^2 - 2 x_t.y_g          (x=targets, y=gen)
  k        = exp(-2.5*sqrt(d2)),  k[diag]=0             (dist normalized by sqrt(D)=8,
                                                         TEMP=0.05 -> exp(-2.5*sqrt(d2)))
  row[g]   = sum_t k[t,g];  col[t] = sum_g k[t,g]
  nk       = k * min(rsqrt(row)*rsqrt(col), 1e6)        (== k / sqrt(max(row*col,1e-12)))
  s_gen[g] = sum_{t<G} nk ; s_pos[g] = sum_{t>=G} nk
  M_pos    = nk[G:].T @ pos ; M_neg = nk[:G].T @ gen    (transposed layout)
  V        = s_gen*M_pos - s_pos*M_neg
  loss    += sum(V^2);  drift += sum_g ||V[g]||

One class per NeuronCore (8 classes / 8 cores), no collectives; host sums
the 8 scalar pairs.

Implementation notes:
  - d2 via Gram matmuls in float32r (fp32 data, 1 cycle/row on PE for
    512-wide outputs vs 4 for plain fp32).
  - 2-pass ACT chain instead of the 3-pass ln/exp/exp: S = sqrt(6.25*d2
    + eps) then k = exp(-S) with column-sum accumulation.  sqrt and exp
    live in different activation-table sets, so chunks are processed in
    groups of GK between table switches (1283ns per load), with S parked
    in a [128, GK, G] f32 scratch.
  - Row sums: pos chunks via ones-matmuls; the gen-gen block is
    symmetric so its row sums equal colacc (transposed into the same
    PSUM accumulators by a colacc x identity matmul).
  - Phase 2 re-reads the persistent bf16 KT: W = min(bcast(rsqrt(row)) *
    rsqrt(col)_t, 1e6) (DVE 4x mode), nk chunk = KT*W (DVE 2x), side
    matmuls psum[side] += aug(side).T @ nk with aug = [x, 1] so row 64
    holds s_gen/s_pos.
  - Diagonal masking: BIG*I matmul accumulated onto the d2 PSUM makes
    k_diag = exp(-sqrt(6.25*BIG)) underflow to 0 in bf16.
"""

import sys

for _p in ("/opt/trn_rl_repo", "/root/.axon_site/_ro/trn_rl_repo"):
    if _p not in sys.path:
        sys.path.insert(0, _p)

import ml_dtypes
import numpy as np

C = 8
BIG = 4000.0  # diag d2 offset: exp(-sqrt(6.25*4000)) = exp(-158) -> 0 in bf16
EPS = 0.01  # sqrt(6.25*d2 + EPS); guards d2 ~ -1e-4 roundoff on the diagonal
GK = 8  # chunks per activation-table group

_CACHE = {}


def _patch_act_tables():
    """Restrict ACT table sets so the load inserter never thrashes: Sqrt
    lives only in sqrt_and_others, Ln/Exp only in natural_log_exp_and_others
    (the default sets split ln and exp across natural_log / exp_and_others,
    which makes every Ln->Exp pair reload a table).  Set IDs stay aligned
    with the compiler's act_info.json because only membership is filtered."""
    import functools

    import concourse.bacc as bacc
    import concourse.hw_specs as hw_specs

    if getattr(hw_specs.get_activation_tables, "_drift_patched", False):
        return
    orig = hw_specs.get_activation_tables

    @functools.cache
    def patched(module_arch):
        keep = ("natural_log_exp_and_others", "sqrt_and_others")
        return {
            name: (funcs if name in keep else set())
            for name, funcs in orig(module_arch).items()
        }

    patched._drift_patched = True
    hw_specs.get_activation_tables = patched
    bacc.get_activation_tables = patched


def _build(G, P):
    import concourse.bacc as bacc
    import concourse.tile as tile
    from concourse import mybir

    _patch_act_tables()

    f32 = mybir.dt.float32
    f32r = mybir.dt.float32r
    f16 = mybir.dt.float16
    bf16 = mybir.dt.bfloat16
    AF = mybir.ActivationFunctionType
    OP = mybir.AluOpType

    T = G + P
    NT = T // 128  # t-chunks (partition dim of KT)
    NG = G // 128  # gen-block t-chunks
    RW = 512  # matmul slice width (psum bank limit: 512 fp32)
    GR = G // RW
    HW = G // 2  # d2 half width (2 psum banks per half)
    assert T % 128 == 0 and G % 512 == 0 and P % 128 == 0 and NT % GK == 0

    nc = bacc.Bacc("TRN2", target_bir_lowering=False, debug=False, num_devices=8)

    ta = nc.dram_tensor("ta", [66, T], f32r, kind="ExternalInput")
    gb = nc.dram_tensor("gb", [66, G], f32r, kind="ExternalInput")
    posa = nc.dram_tensor("posa", [128, P // 128, 65], bf16, kind="ExternalInput")
    gena = nc.dram_tensor("gena", [128, G // 128, 65], bf16, kind="ExternalInput")
    ident = nc.dram_tensor("ident", [128, 128], bf16, kind="ExternalInput")
    identf = nc.dram_tensor("identf", [128, 128], f32, kind="ExternalInput")
    bigi = nc.dram_tensor("bigi", [128, 128], bf16, kind="ExternalInput")
    ones_r = nc.dram_tensor("ones_r", [128, 1], bf16, kind="ExternalInput")
    ones_b = nc.dram_tensor("ones_b", [1, 128], f32, kind="ExternalInput")
    ones_v = nc.dram_tensor("ones_v", [64, 1], f32, kind="ExternalInput")
    outh = nc.dram_tensor("out", [1, 2], f32, kind="ExternalOutput")

    from contextlib import ExitStack

    with tile.TileContext(nc) as tc, ExitStack() as ctx:
        kpool = ctx.enter_context(tc.tile_pool(name="kpool", bufs=1))
        bigp = ctx.enter_context(tc.tile_pool(name="bigp", bufs=1))
        singles = ctx.enter_context(tc.tile_pool(name="singles", bufs=1))
        smalls = ctx.enter_context(tc.tile_pool(name="smalls", bufs=1))

        # persistent kernel matrix, [128, NT, G] bf16 (t-chunk major)
        KT = kpool.tile([128, NT, G], bf16)

        POSAs = singles.tile([128, P // 128, 65], bf16)
        GENAs = singles.tile([128, G // 128, 65], bf16)
        IDENTs = singles.tile([128, 128], bf16)
        IDENTFs = singles.tile([128, 128], f32)
        BIGIs = singles.tile([128, 128], bf16)
        ONESRs = singles.tile([128, 1], bf16)
        ONESBs = singles.tile([1, 128], f32)
        ONESVs = singles.tile([64, 1], f32)

        epsb = smalls.tile([128, 1], f32)
        qeps = smalls.tile([1, 1], f32)
        colacc = smalls.tile([128, NT], f32)
        lnc = smalls.tile([128, NT], f32)
        bvec = smalls.tile([128, NT], f32)
        lsums = smalls.tile([1, GR], f32)
        dsums = smalls.tile([1, GR], f32)

        rowS = bigp.tile([1, G], f32, tag="rowS")
        BCA = bigp.tile([128, G], bf16, tag="bca")

        order = list(range(NG, NT)) + list(range(NG))  # pos chunks first

        # phase-1-only tensors live in their own pool, freed before phase 2
        with tc.tile_pool(name="ph1", bufs=1) as p1:
            TAs = p1.tile([66, T], f32r, tag="ta")
            GBs = p1.tile([66, G], f32r, tag="gb")
            S = p1.tile([128, GK, G], f16, tag="s")  # sqrt scratch (fp16: ~5e-4 rel on S)

            # DMA order: the chunk loop starts at the pos block (i=NG), so
            # its TA quarter + GB must land first; late-needed tensors ride
            # the queue tails.  ACT queue stays DMA-free (first instruction
            # must be the sqrt table load).
            h, q3 = T // 2, 3 * T // 4
            nc.sync.dma_start(out=TAs[:, h : h + 256], in_=ta[:, h : h + 256])
            nc.gpsimd.dma_start(out=GBs[:, :512], in_=gb[:, :512])
            nc.sync.dma_start(out=TAs[:, h + 256 : q3], in_=ta[:, h + 256 : q3])
            nc.sync.dma_start(out=TAs[:, q3:], in_=ta[:, q3:])
            nc.gpsimd.dma_start(out=GBs[:, 512 : G // 2], in_=gb[:, 512 : G // 2])
            nc.gpsimd.dma_start(out=GBs[:, G // 2 :], in_=gb[:, G // 2 :])
            nc.gpsimd.dma_start(out=ONESRs[:], in_=ones_r[:, :])
            nc.gpsimd.dma_start(out=POSAs[:], in_=posa[:, :, :])
            nc.gpsimd.dma_start(out=GENAs[:], in_=gena[:, :, :])
            nc.sync.dma_start(out=IDENTs[:], in_=ident[:, :])
            nc.sync.dma_start(out=IDENTFs[:], in_=identf[:, :])
            nc.sync.dma_start(out=BIGIs[:], in_=bigi[:, :])
            nc.sync.dma_start(out=ONESBs[:], in_=ones_b[:, :])
            nc.sync.dma_start(out=ONESVs[:], in_=ones_v[:, :])
            nc.sync.dma_start(out=TAs[:, :h], in_=ta[:, :h])

            nc.vector.memset(epsb[:], EPS)
            nc.vector.memset(qeps[:], 1.0e-35)

            groups = [order[g * GK : (g + 1) * GK] for g in range(NT // GK)]

            # ---- Phase 1: build KT (sqrt/exp table groups), col + row sums
            with (
                tc.tile_pool(name="d2p", bufs=1, space="PSUM") as dp,
                tc.tile_pool(name="rap", bufs=1, space="PSUM") as rp,
            ):
                rowaccs = [
                    rp.tile([1, RW], f32, tag=f"ra{j}", name=f"rowacc{j}")
                    for j in range(GR)
                ]
                for grp in groups:
                    # sqrt phase: d2 matmuls ping-pong 2 psum half-tiles
                    for slot, i in enumerate(grp):
                        jd = i // (RW // 128) if i < NG else -1
                        for hh in range(2):
                            d2 = dp.tile(
                                [128, HW], f32, tag=f"d2{hh}", name=f"d2{hh}"
                            )
                            for jj in range(HW // RW):
                                j = hh * (HW // RW) + jj
                                nc.tensor.matmul(
                                    d2[:, jj * RW : (jj + 1) * RW],
                                    TAs[:, i * 128 : (i + 1) * 128],
                                    GBs[:, j * RW : (j + 1) * RW],
                                    start=True,
                                    stop=(j != jd),
                                    skip_group_check=True,
                                )
                                if j == jd:
                                    c0 = i * 128 - hh * HW
                                    nc.tensor.matmul(
                                        d2[:, c0 : c0 + 128],
                                        IDENTs[:],
                                        BIGIs[:],
                                        start=False,
                                        stop=True,
                                        skip_group_check=True,
                                    )
                            nc.scalar.activation(
                                S[:, slot, hh * HW : (hh + 1) * HW],
                                d2[:],
                                AF.Sqrt,
                                bias=epsb[:],
                                scale=6.25,
                            )
                    # exp phase: k chunks -> KT + col sums; row-sum matmuls
                    for slot, i in enumerate(grp):
                        nc.scalar.activation(
                            KT[:, i, :], S[:, slot, :], AF.Exp, scale=-1.0,
                            accum_out=colacc[:, i : i + 1],
                        )
                        if i < NG:
                            # gen-gen block symmetric: its row sums ==
                            # colacc; transpose into the rowacc banks
                            j, m = divmod(i * 128, RW)
                            nc.tensor.matmul(
                                rowaccs[j][:, m : m + 128],
                                colacc[:, i : i + 1],
                                IDENTFs[:],
                                start=False,
                                stop=(m + 128 == RW),
                                skip_group_check=True,
                            )
                        else:
                            for j in range(GR):
                                nc.tensor.matmul(
                                    rowaccs[j][:],
                                    ONESRs[:],
                                    KT[:, i, j * RW : (j + 1) * RW],
                                    start=(i == NG),
                                    stop=False,
                                    skip_group_check=True,
                                )
                # a = rsqrt(row) straight from PSUM (exp/ln table is loaded)
                for j in range(GR):
                    js = slice(j * RW, (j + 1) * RW)
                    nc.scalar.activation(rowS[:, js], rowaccs[j][:], AF.Ln)
                    nc.scalar.activation(
                        rowS[:, js], rowS[:, js], AF.Exp, scale=-0.5
                    )

            # b = rsqrt(col) per t (per-partition, [128, NT])
            nc.scalar.activation(lnc[:], colacc[:], AF.Ln)
            nc.scalar.activation(bvec[:], lnc[:], AF.Exp, scale=-0.5)

            # broadcast a across partitions -> [128, G] bf16
            with tc.tile_pool(name="bcp", bufs=2, space="PSUM") as bp:
                for j in range(GR):
                    pb = bp.tile([128, RW], f32, tag="bc")
                    nc.tensor.matmul(
                        pb[:], ONESBs[:], rowS[:, j * RW : (j + 1) * RW],
                        start=True, stop=True, skip_group_check=True,
                    )
                    nc.vector.tensor_copy(BCA[:, j * RW : (j + 1) * RW], pb[:])

        # ---- Phase 2: nk chunks = KT * min(a*b, 1e6), side matmuls
        # (ph2 pool opens after ph1 closed, reusing its SBUF region)
        ph2 = ctx.enter_context(tc.tile_pool(name="ph2", bufs=1))
        PNo = ph2.tile([65, G], f32, tag="pn")
        PPo = ph2.tile([65, G], f32, tag="pp")
        with (
            tc.tile_pool(name="wp", bufs=2) as wp,
            tc.tile_pool(name="nkp", bufs=3) as nkp,
            tc.tile_pool(name="sp2", bufs=1, space="PSUM") as p2,
        ):
            psums = None
            for i in order:
                if i in (NG, 0):  # entering pos / gen half: (re)use banks
                    psums = [
                        p2.tile([65, RW], f32, tag=f"sp{j}", name=f"sp{j}_{i}")
                        for j in range(GR)
                    ]
                W = wp.tile([128, G], bf16, tag="w")
                nc.vector.tensor_scalar(
                    W[:], BCA[:], bvec[:, i : i + 1], 1.0e6,
                    op0=OP.mult, op1=OP.min,
                )
                NK = nkp.tile([128, G], bf16, tag="nk")
                # DVE is the phase-2 bottleneck; GpSimd runs tensor_tensor
                # at ~1.7us vs DVE's ~0.9us, so give it 5 of every 8 chunks
                eng = nc.gpsimd if i % 8 < 5 else nc.vector
                eng.tensor_mul(NK[:], KT[:, i, :], W[:])
                side = 0 if i < NG else 1
                lhs = GENAs[:, i, :] if side == 0 else POSAs[:, i - NG, :]
                first = i in (0, NG)
                last = i in (NG - 1, NT - 1)
                for j in range(GR):
                    nc.tensor.matmul(
                        psums[j][:],
                        lhs,
                        NK[:, j * RW : (j + 1) * RW],
                        start=first,
                        stop=last,
                        skip_group_check=True,
                    )
                if i == NT - 1:  # pos side complete -> drain
                    for j in range(GR):
                        nc.vector.tensor_copy(
                            PPo[:, j * RW : (j + 1) * RW], psums[j][:]
                        )
                if i == NG - 1:  # gen side complete -> drain
                    for j in range(GR):
                        nc.vector.tensor_copy(
                            PNo[:, j * RW : (j + 1) * RW], psums[j][:]
                        )

        # ---- tail: V.T = bcast(s_gen)*M_pos.T - bcast(s_pos)*M_neg.T
        # rows 0:64 of PNo/PPo = M_neg.T / M_pos.T ; row 64 = s_gen / s_pos
        qS = bigp.tile([1, G], f32, tag="rowS")  # reuse the rowS region
        with (
            tc.tile_pool(name="vtp", bufs=1) as vtp,
            tc.tile_pool(name="bc2", bufs=2, space="PSUM") as bp2,
            tc.tile_pool(name="qp", bufs=2, space="PSUM") as qp,
        ):
            for j in range(GR):
                js = slice(j * RW, (j + 1) * RW)
                sgr = vtp.tile([1, RW], f32, tag="sgr")
                spr = vtp.tile([1, RW], f32, tag="spr")
                nc.vector.tensor_copy(sgr[:], PNo[64:65, js])
                nc.vector.tensor_copy(spr[:], PPo[64:65, js])
                bg = bp2.tile([64, RW], f32, tag="bg")
                bpp = bp2.tile([64, RW], f32, tag="bp")
                nc.tensor.matmul(
                    bg[:], ONESBs[:, 0:64], sgr[:],
                    start=True, stop=True, skip_group_check=True,
                )
                nc.tensor.matmul(
                    bpp[:], ONESBs[:, 0:64], spr[:],
                    start=True, stop=True, skip_group_check=True,
                )
                vt1 = vtp.tile([64, RW], f32, tag="vt1", bufs=2)
                vt2 = vtp.tile([64, RW], f32, tag="vt2", bufs=2)
                nc.vector.tensor_mul(vt1[:], PPo[0:64, js], bg[:])
                nc.vector.tensor_mul(vt2[:], PNo[0:64, js], bpp[:])
                nc.vector.tensor_sub(vt1[:], vt1[:], vt2[:])
                nc.vector.tensor_mul(vt2[:], vt1[:], vt1[:])  # V^2
                qt = qp.tile([1, RW], f32, tag="q")
                nc.tensor.matmul(
                    qt[:], ONESVs[:], vt2[:], start=True, stop=True,
                    skip_group_check=True,
                )
                # loss partial = sum(q) via copy-with-accum; drift partial =
                # sum(sqrt(q)) via ln/exp with accum
                nc.vector.tensor_scalar(
                    qS[:, js], qt[:], 1.0, 0.0, op0=OP.mult, op1=OP.add,
                    accum_out=lsums[:, j : j + 1],
                )
                nc.scalar.activation(qS[:, js], qS[:, js], AF.Ln, bias=qeps[:])
                nc.scalar.activation(
                    qS[:, js], qS[:, js], AF.Exp, scale=0.5,
                    accum_out=dsums[:, j : j + 1],
                )

        outS = smalls.tile([1, 2], f32)
        nc.vector.tensor_reduce(
            outS[:, 0:1], lsums[:], axis=mybir.AxisListType.X, op=OP.add
        )
        nc.vector.tensor_reduce(
            outS[:, 1:2], dsums[:], axis=mybir.AxisListType.X, op=OP.add
        )
        nc.sync.dma_start(out=outh[:, :], in_=outS[:])

    nc.compile()
    return nc


def _prep_class(gen_c, pos_c):
    """Host-side input prep for one class -> dict of named arrays."""
    gen_c = np.ascontiguousarray(gen_c, dtype=np.float32)
    pos_c = np.ascontiguousarray(pos_c, dtype=np.float32)
    G, D = gen_c.shape
    targets = np.concatenate([gen_c, pos_c], axis=0)
    T = targets.shape[0]

    ta = np.empty((66, T), np.float32)
    ta[0:64] = -2.0 * targets.T
    ta[64] = (targets * targets).sum(axis=1)
    ta[65] = 1.0

    gbm = np.empty((66, G), np.float32)
    gbm[0:64] = gen_c.T
    gbm[64] = 1.0
    gbm[65] = (gen_c * gen_c).sum(axis=1)

    def aug(x):
        n = x.shape[0]
        a = np.empty((n, 65), np.float32)
        a[:, 0:64] = x
        a[:, 64] = 1.0
        return (
            a.astype(ml_dtypes.bfloat16)
            .reshape(n // 128, 128, 65)
            .transpose(1, 0, 2)
            .copy()
        )

    bf = ml_dtypes.bfloat16
    return {
        "ta": ta,
        "gb": gbm,
        "posa": aug(pos_c),
        "gena": aug(gen_c),
        "ident": np.eye(128, dtype=bf),
        "identf": np.eye(128, dtype=np.float32),
        "bigi": (BIG * np.eye(128)).astype(bf),
        "ones_r": np.ones((128, 1), bf),
        "ones_b": np.ones((1, 128), np.float32),
        "ones_v": np.ones((64, 1), np.float32),
    }


def kernel(generated, labels_gen, positive, labels_pos):
    from concourse.bass_utils import run_bass_kernel_spmd

    generated = np.asarray(generated, dtype=np.float32)
    positive = np.asarray(positive, dtype=np.float32)
    N, D = generated.shape
    Np = positive.shape[0]
    G, P = N // C, Np // C
    assert D == 64

    key = (G, P)
    if key not in _CACHE:
        _CACHE[key] = _build(G, P)
    nc = _CACHE[key]

    in_maps = [
        _prep_class(
            generated[c * G : (c + 1) * G], positive[c * P : (c + 1) * P]
        )
        for c in range(C)
    ]
    res = run_bass_kernel_spmd(nc, in_maps, core_ids=list(range(C)))
    sums = np.stack([res.results[i]["out"][0] for i in range(C)])  # [C, 2]
    loss = sums[:, 0].sum() / (N * D)
    dn = sums[:, 1].sum() / N
    return np.float32(loss), np.float32(dn)


if __name__ == "__main__":
    rng = np.random.default_rng(0)
    N = 16384
    gen = rng.standard_normal((N, 64), dtype=np.float32)
    pos = rng.standard_normal((N, 64), dtype=np.float32)
    lg = np.repeat(np.arange(C), N // C).astype(np.int32)
    print(kernel(gen, lg, pos, lg))


# revision 27
# speedup vs baseline: 1.2893x; 1.0264x over previous
"""Trainium2 Bass kernel for nn_ClassConditionalDriftingLoss.

Math per class c (G gen rows, P pos rows, D=64, T=G+P targets):
  d2[t,g]  = ||x_t||^2 + ||y_g||^2 - 2 x_t.y_g          (x=targets, y=gen)
  k        = exp(-2.5*sqrt(d2)),  k[diag]=0             (dist normalized by sqrt(D)=8,
                                                         TEMP=0.05 -> exp(-2.5*sqrt(d2)))
  row[g]   = sum_t k[t,g];  col[t] = sum_g k[t,g]
  nk       = k * min(rsqrt(row)*rsqrt(col), 1e6)        (== k / sqrt(max(row*col,1e-12)))
  s_gen[g] = sum_{t<G} nk ; s_pos[g] = sum_{t>=G} nk
  M_pos    = nk[G:].T @ pos ; M_neg = nk[:G].T @ gen    (transposed layout)
  V        = s_gen*M_pos - s_pos*M_neg
  loss    += sum(V^2);  drift += sum_g ||V[g]||

One class per NeuronCore (8 classes / 8 cores), no collectives; host sums
the 8 scalar pairs.

Implementation notes:
  - d2 via Gram matmuls in float32r (fp32 data, 1 cycle/row on PE for
    512-wide outputs vs 4 for plain fp32).
  - 2-pass ACT chain instead of the 3-pass ln/exp/exp: S = sqrt(6.25*d2
    + eps) then k = exp(-S) with column-sum accumulation.  sqrt and exp
    live in different activation-table sets, so chunks are processed in
    groups of GK between table switches (1283ns per load), with S parked
    in a [128, GK, G] f32 scratch.
  - Row sums: pos chunks via ones-matmuls; the gen-gen block is
    symmetric so its row sums equal colacc (transposed into the same
    PSUM accumulators by a colacc x identity matmul).
  - Phase 2 re-reads the persistent bf16 KT: W = min(bcast(rsqrt(row)) *
    rsqrt(col)_t, 1e6) (DVE 4x mode), nk chunk = KT*W (DVE 2x), side
    matmuls psum[side] += aug(side).T @ nk with aug = [x, 1] so row 64
    holds s_gen/s_pos.
  - Diagonal masking: BIG*I matmul accumulated onto the d2 PSUM makes
    k_diag = exp(-sqrt(6.25*BIG)) underflow to 0 in bf16.
"""

import sys

for _p in ("/opt/trn_rl_repo", "/root/.axon_site/_ro/trn_rl_repo"):
    if _p not in sys.path:
        sys.path.insert(0, _p)

import ml_dtypes
import numpy as np

C = 8
BIG = 4000.0  # diag d2 offset: exp(-sqrt(6.25*4000)) = exp(-158) -> 0 in bf16
EPS = 0.01  # sqrt(6.25*d2 + EPS); guards d2 ~ -1e-4 roundoff on the diagonal
GK = 8  # chunks per activation-table group

_CACHE = {}


def _patch_act_tables():
    """Restrict ACT table sets so the load inserter never thrashes: Sqrt
    lives only in sqrt_and_others, Ln/Exp only in natural_log_exp_and_others
    (the default sets split ln and exp across natural_log / exp_and_others,
    which makes every Ln->Exp pair reload a table).  Set IDs stay aligned
    with the compiler's act_info.json because only membership is filtered."""
    import functools

    import concourse.bacc as bacc
    import concourse.hw_specs as hw_specs

    if getattr(hw_specs.get_activation_tables, "_drift_patched", False):
        return
    orig = hw_specs.get_activation_tables

    @functools.cache
    def patched(module_arch):
        keep = ("natural_log_exp_and_others", "sqrt_and_others")
        return {
            name: (funcs if name in keep else set())
            for name, funcs in orig(module_arch).items()
        }

    patched._drift_patched = True
    hw_specs.get_activation_tables = patched
    bacc.get_activation_tables = patched


def _build(G, P):
    import concourse.bacc as bacc
    import concourse.tile as tile
    from concourse import mybir

    _patch_act_tables()

    f32 = mybir.dt.float32
    f32r = mybir.dt.float32r
    f16 = mybir.dt.float16
    bf16 = mybir.dt.bfloat16
    AF = mybir.ActivationFunctionType
    OP = mybir.AluOpType

    T = G + P
    NT = T // 128  # t-chunks (partition dim of KT)
    NG = G // 128  # gen-block t-chunks
    RW = 512  # matmul slice width (psum bank limit: 512 fp32)
    GR = G // RW
    HW = G // 2  # d2 half width (2 psum banks per half)
    assert T % 128 == 0 and G % 512 == 0 and P % 128 == 0 and NT % GK == 0

    nc = bacc.Bacc("TRN2", target_bir_lowering=False, debug=False, num_devices=8)

    ta = nc.dram_tensor("ta", [66, T], f32r, kind="ExternalInput")
    gb = nc.dram_tensor("gb", [66, G], f32r, kind="ExternalInput")
    posa = nc.dram_tensor("posa", [128, P // 128, 65], bf16, kind="ExternalInput")
    gena = nc.dram_tensor("gena", [128, G // 128, 65], bf16, kind="ExternalInput")
    ident = nc.dram_tensor("ident", [128, 128], bf16, kind="ExternalInput")
    identf = nc.dram_tensor("identf", [128, 128], f32, kind="ExternalInput")
    bigi = nc.dram_tensor("bigi", [128, 128], bf16, kind="ExternalInput")
    ones_r = nc.dram_tensor("ones_r", [128, 1], bf16, kind="ExternalInput")
    ones_b = nc.dram_tensor("ones_b", [1, 128], f32, kind="ExternalInput")
    ones_v = nc.dram_tensor("ones_v", [64, 1], f32, kind="ExternalInput")
    outh = nc.dram_tensor("out", [1, 2], f32, kind="ExternalOutput")

    from contextlib import ExitStack

    with tile.TileContext(nc) as tc, ExitStack() as ctx:
        kpool = ctx.enter_context(tc.tile_pool(name="kpool", bufs=1))
        bigp = ctx.enter_context(tc.tile_pool(name="bigp", bufs=1))
        singles = ctx.enter_context(tc.tile_pool(name="singles", bufs=1))
        smalls = ctx.enter_context(tc.tile_pool(name="smalls", bufs=1))

        # persistent kernel matrix, [128, NT, G] bf16 (t-chunk major)
        KT = kpool.tile([128, NT, G], bf16)

        POSAs = singles.tile([128, P // 128, 65], bf16)
        GENAs = singles.tile([128, G // 128, 65], bf16)
        IDENTs = singles.tile([128, 128], bf16)
        IDENTFs = singles.tile([128, 128], f32)
        BIGIs = singles.tile([128, 128], bf16)
        ONESRs = singles.tile([128, 1], bf16)
        ONESBs = singles.tile([1, 128], f32)
        ONESVs = singles.tile([64, 1], f32)

        epsb = smalls.tile([128, 1], f32)
        qeps = smalls.tile([1, 1], f32)
        colacc = smalls.tile([128, NT], f32)
        lnc = smalls.tile([128, NT], f32)
        bvec = smalls.tile([128, NT], f32)
        lsums = smalls.tile([1, GR], f32)
        dsums = smalls.tile([1, GR], f32)

        rowS = bigp.tile([1, G], f32, tag="rowS")
        BCA = bigp.tile([128, G], bf16, tag="bca")

        order = list(range(NG, NT)) + list(range(NG))  # pos chunks first

        # phase-1-only tensors live in their own pool, freed before phase 2
        with tc.tile_pool(name="ph1", bufs=1) as p1:
            TAs = p1.tile([66, T], f32r, tag="ta")
            GBs = p1.tile([66, G], f32r, tag="gb")
            S = p1.tile([128, GK, G], f16, tag="s")  # sqrt scratch (fp16: ~5e-4 rel on S)

            # DMA order: the chunk loop starts at the pos block (i=NG), so
            # its TA quarter + GB must land first; late-needed tensors ride
            # the queue tails.  ACT queue stays DMA-free (first instruction
            # must be the sqrt table load).
            h, q3 = T // 2, 3 * T // 4
            nc.sync.dma_start(out=TAs[:, h : h + 256], in_=ta[:, h : h + 256])
            nc.gpsimd.dma_start(out=GBs[:, :512], in_=gb[:, :512])
            nc.sync.dma_start(out=TAs[:, h + 256 : q3], in_=ta[:, h + 256 : q3])
            nc.sync.dma_start(out=TAs[:, q3:], in_=ta[:, q3:])
            nc.gpsimd.dma_start(out=GBs[:, 512 : G // 2], in_=gb[:, 512 : G // 2])
            nc.gpsimd.dma_start(out=GBs[:, G // 2 :], in_=gb[:, G // 2 :])
            nc.gpsimd.dma_start(out=ONESRs[:], in_=ones_r[:, :])
            nc.gpsimd.dma_start(out=POSAs[:], in_=posa[:, :, :])
            nc.gpsimd.dma_start(out=GENAs[:], in_=gena[:, :, :])
            nc.sync.dma_start(out=IDENTs[:], in_=ident[:, :])
            nc.sync.dma_start(out=IDENTFs[:], in_=identf[:, :])
            nc.sync.dma_start(out=BIGIs[:], in_=bigi[:, :])
            nc.sync.dma_start(out=ONESBs[:], in_=ones_b[:, :])
            nc.sync.dma_start(out=ONESVs[:], in_=ones_v[:, :])
            nc.sync.dma_start(out=TAs[:, :h], in_=ta[:, :h])

            nc.vector.memset(epsb[:], EPS)
            nc.vector.memset(qeps[:], 1.0e-35)

            groups = [order[g * GK : (g + 1) * GK] for g in range(NT // GK)]

            # ---- Phase 1: build KT (sqrt/exp table groups), col + row sums
            with (
                tc.tile_pool(name="d2p", bufs=1, space="PSUM") as dp,
                tc.tile_pool(name="rap", bufs=1, space="PSUM") as rp,
            ):
                rowaccs = [
                    rp.tile([1, RW], f32, tag=f"ra{j}", name=f"rowacc{j}")
                    for j in range(GR)
                ]
                for grp in groups:
                    # sqrt phase: d2 matmuls ping-pong 2 psum half-tiles
                    for slot, i in enumerate(grp):
                        jd = i // (RW // 128) if i < NG else -1
                        for hh in range(2):
                            d2 = dp.tile(
                                [128, HW], f32, tag=f"d2{hh}", name=f"d2{hh}"
                            )
                            for jj in range(HW // RW):
                                j = hh * (HW // RW) + jj
                                nc.tensor.matmul(
                                    d2[:, jj * RW : (jj + 1) * RW],
                                    TAs[:, i * 128 : (i + 1) * 128],
                                    GBs[:, j * RW : (j + 1) * RW],
                                    start=True,
                                    stop=(j != jd),
                                    skip_group_check=True,
                                )
                                if j == jd:
                                    c0 = i * 128 - hh * HW
                                    nc.tensor.matmul(
                                        d2[:, c0 : c0 + 128],
                                        IDENTs[:],
                                        BIGIs[:],
                                        start=False,
                                        stop=True,
                                        skip_group_check=True,
                                    )
                            nc.scalar.activation(
                                S[:, slot, hh * HW : (hh + 1) * HW],
                                d2[:],
                                AF.Sqrt,
                                bias=epsb[:],
                                scale=6.25,
                            )
                    # exp phase: k chunks -> KT + col sums; row-sum matmuls
                    for slot, i in enumerate(grp):
                        nc.scalar.activation(
                            KT[:, i, :], S[:, slot, :], AF.Exp, scale=-1.0,
                            accum_out=colacc[:, i : i + 1],
                        )
                        if i < NG:
                            # gen-gen block symmetric: its row sums ==
                            # colacc; transpose into the rowacc banks
                            j, m = divmod(i * 128, RW)
                            nc.tensor.matmul(
                                rowaccs[j][:, m : m + 128],
                                colacc[:, i : i + 1],
                                IDENTFs[:],
                                start=False,
                                stop=(m + 128 == RW),
                                skip_group_check=True,
                            )
                            if m + 128 == RW:
                                # slice j's row sums are complete: fold in
                                # a=rsqrt(row) and broadcast to BCA now,
                                # overlapped under the remaining chunks
                                js = slice(j * RW, (j + 1) * RW)
                                nc.scalar.activation(
                                    rowS[:, js], rowaccs[j][:], AF.Ln
                                )
                                nc.scalar.activation(
                                    rowS[:, js], rowS[:, js], AF.Exp,
                                    scale=-0.5,
                                )
                                pb = rp.tile(
                                    [128, RW], f32, tag=f"ra{j}",
                                    name=f"bc{j}",
                                )
                                nc.tensor.matmul(
                                    pb[:], ONESBs[:], rowS[:, js],
                                    start=True, stop=True,
                                    skip_group_check=True,
                                )
                                nc.vector.tensor_copy(BCA[:, js], pb[:])
                        else:
                            for j in range(GR):
                                nc.tensor.matmul(
                                    rowaccs[j][:],
                                    ONESRs[:],
                                    KT[:, i, j * RW : (j + 1) * RW],
                                    start=(i == NG),
                                    stop=False,
                                    skip_group_check=True,
                                )
            # b = rsqrt(col) per t (per-partition, [128, NT])
            nc.scalar.activation(lnc[:], colacc[:], AF.Ln)
            nc.scalar.activation(bvec[:], lnc[:], AF.Exp, scale=-0.5)

        # ---- Phase 2: nk chunks = KT * min(a*b, 1e6), side matmuls
        # (ph2 pool opens after ph1 closed, reusing its SBUF region)
        ph2 = ctx.enter_context(tc.tile_pool(name="ph2", bufs=1))
        PNo = ph2.tile([65, G], f32, tag="pn")
        PPo = ph2.tile([65, G], f32, tag="pp")
        with (
            tc.tile_pool(name="wp", bufs=2) as wp,
            tc.tile_pool(name="nkp", bufs=3) as nkp,
            tc.tile_pool(name="sp2", bufs=1, space="PSUM") as p2,
        ):
            psums = None
            for i in order:
                if i in (NG, 0):  # entering pos / gen half: (re)use banks
                    psums = [
                        p2.tile([65, RW], f32, tag=f"sp{j}", name=f"sp{j}_{i}")
                        for j in range(GR)
                    ]
                W = wp.tile([128, G], bf16, tag="w")
                nc.vector.tensor_scalar(
                    W[:], BCA[:], bvec[:, i : i + 1], 1.0e6,
                    op0=OP.mult, op1=OP.min,
                )
                NK = nkp.tile([128, G], bf16, tag="nk")
                # DVE is the phase-2 bottleneck; GpSimd runs tensor_tensor
                # at ~1.7us vs DVE's ~0.9us, so give it 5 of every 8 chunks
                eng = nc.gpsimd if i % 8 in (0, 2, 3, 5, 7) else nc.vector
                eng.tensor_mul(NK[:], KT[:, i, :], W[:])
                side = 0 if i < NG else 1
                lhs = GENAs[:, i, :] if side == 0 else POSAs[:, i - NG, :]
                first = i in (0, NG)
                last = i in (NG - 1, NT - 1)
                for j in range(GR):
                    nc.tensor.matmul(
                        psums[j][:],
                        lhs,
                        NK[:, j * RW : (j + 1) * RW],
                        start=first,
                        stop=last,
                        skip_group_check=True,
                    )
                if i == NT - 1:  # pos side complete -> drain (ACT is idle)
                    for j in range(GR):
                        nc.scalar.activation(
                            PPo[:, j * RW : (j + 1) * RW], psums[j][:], AF.Copy
                        )
                if i == NG - 1:  # gen side complete -> drain
                    for j in range(GR):
                        nc.scalar.activation(
                            PNo[:, j * RW : (j + 1) * RW], psums[j][:], AF.Copy
                        )

        # ---- tail: V.T = bcast(s_gen)*M_pos.T - bcast(s_pos)*M_neg.T
        # rows 0:64 of PNo/PPo = M_neg.T / M_pos.T ; row 64 = s_gen / s_pos
        qS = bigp.tile([1, G], f32, tag="rowS")  # reuse the rowS region
        with (
            tc.tile_pool(name="vtp", bufs=1) as vtp,
            tc.tile_pool(name="bc2", bufs=2, space="PSUM") as bp2,
            tc.tile_pool(name="qp", bufs=2, space="PSUM") as qp,
        ):
            for j in range(GR):
                js = slice(j * RW, (j + 1) * RW)
                sgr = vtp.tile([1, RW], f32, tag="sgr")
                spr = vtp.tile([1, RW], f32, tag="spr")
                nc.vector.tensor_copy(sgr[:], PNo[64:65, js])
                nc.vector.tensor_copy(spr[:], PPo[64:65, js])
                bg = bp2.tile([64, RW], f32, tag="bg")
                bpp = bp2.tile([64, RW], f32, tag="bp")
                nc.tensor.matmul(
                    bg[:], ONESBs[:, 0:64], sgr[:],
                    start=True, stop=True, skip_group_check=True,
                )
                nc.tensor.matmul(
                    bpp[:], ONESBs[:, 0:64], spr[:],
                    start=True, stop=True, skip_group_check=True,
                )
                vt1 = vtp.tile([64, RW], f32, tag="vt1", bufs=2)
                vt2 = vtp.tile([64, RW], f32, tag="vt2", bufs=2)
                nc.vector.tensor_mul(vt1[:], PPo[0:64, js], bg[:])
                nc.vector.tensor_mul(vt2[:], PNo[0:64, js], bpp[:])
                nc.vector.tensor_sub(vt1[:], vt1[:], vt2[:])
                nc.gpsimd.tensor_mul(vt2[:], vt1[:], vt1[:])  # V^2
                qt = qp.tile([1, RW], f32, tag="q")
                nc.tensor.matmul(
                    qt[:], ONESVs[:], vt2[:], start=True, stop=True,
                    skip_group_check=True,
                )
                # loss partial = sum(q) via copy-with-accum; drift partial =
                # sum(sqrt(q)) via ln/exp with accum
                nc.vector.tensor_scalar(
                    qS[:, js], qt[:], 1.0, 0.0, op0=OP.mult, op1=OP.add,
                    accum_out=lsums[:, j : j + 1],
                )
                nc.scalar.activation(qS[:, js], qS[:, js], AF.Ln, bias=qeps[:])
                nc.scalar.activation(
                    qS[:, js], qS[:, js], AF.Exp, scale=0.5,
                    accum_out=dsums[:, j : j + 1],
                )

        outS = smalls.tile([1, 2], f32)
        nc.vector.tensor_reduce(
            outS[:, 0:1], lsums[:], axis=mybir.AxisListType.X, op=OP.add
        )
        nc.vector.tensor_reduce(
            outS[:, 1:2], dsums[:], axis=mybir.AxisListType.X, op=OP.add
        )
        nc.sync.dma_start(out=outh[:, :], in_=outS[:])

    nc.compile()
    return nc


def _prep_class(gen_c, pos_c):
    """Host-side input prep for one class -> dict of named arrays."""
    gen_c = np.ascontiguousarray(gen_c, dtype=np.float32)
    pos_c = np.ascontiguousarray(pos_c, dtype=np.float32)
    G, D = gen_c.shape
    targets = np.concatenate([gen_c, pos_c], axis=0)
    T = targets.shape[0]

    ta = np.empty((66, T), np.float32)
    ta[0:64] = -2.0 * targets.T
    ta[64] = (targets * targets).sum(axis=1)
    ta[65] = 1.0

    gbm = np.empty((66, G), np.float32)
    gbm[0:64] = gen_c.T
    gbm[64] = 1.0
    gbm[65] = (gen_c * gen_c).sum(axis=1)

    def aug(x):
        n = x.shape[0]
        a = np.empty((n, 65), np.float32)
        a[:, 0:64] = x
        a[:, 64] = 1.0
        return (
            a.astype(ml_dtypes.bfloat16)
            .reshape(n // 128, 128, 65)
            .transpose(1, 0, 2)
            .copy()
        )

    bf = ml_dtypes.bfloat16
    return {
        "ta": ta,
        "gb": gbm,
        "posa": aug(pos_c),
        "gena": aug(gen_c),
        "ident": np.eye(128, dtype=bf),
        "identf": np.eye(128, dtype=np.float32),
        "bigi": (BIG * np.eye(128)).astype(bf),
        "ones_r": np.ones((128, 1), bf),
        "ones_b": np.ones((1, 128), np.float32),
        "ones_v": np.ones((64, 1), np.float32),
    }


def kernel(generated, labels_gen, positive, labels_pos):
    from concourse.bass_utils import run_bass_kernel_spmd

    generated = np.asarray(generated, dtype=np.float32)
    positive = np.asarray(positive, dtype=np.float32)
    N, D = generated.shape
    Np = positive.shape[0]
    G, P = N // C, Np // C
    assert D == 64

    key = (G, P)
    if key not in _CACHE:
        _CACHE[key] = _build(G, P)
    nc = _CACHE[key]

    in_maps = [
        _prep_class(
            generated[c * G : (c + 1) * G], positive[c * P : (c + 1) * P]
        )
        for c in range(C)
    ]
    res = run_bass_kernel_spmd(nc, in_maps, core_ids=list(range(C)))
    sums = np.stack([res.results[i]["out"][0] for i in range(C)])  # [C, 2]
    loss = sums[:, 0].sum() / (N * D)
    dn = sums[:, 1].sum() / N
    return np.float32(loss), np.float32(dn)


if __name__ == "__main__":
    rng = np.random.default_rng(0)
    N = 16384
    gen = rng.standard_normal((N, 64), dtype=np.float32)
    pos = rng.standard_normal((N, 64), dtype=np.float32)
    lg = np.repeat(np.arange(C), N // C).astype(np.int32)
    print(kernel(gen, lg, pos, lg))


# revision 29
# speedup vs baseline: 1.3184x; 1.0225x over previous
"""Trainium2 Bass kernel for nn_ClassConditionalDriftingLoss.

Math per class c (G gen rows, P pos rows, D=64, T=G+P targets):
  d2[t,g]  = ||x_t||^2 + ||y_g||^2 - 2 x_t.y_g          (x=targets, y=gen)
  k        = exp(-2.5*sqrt(d2)),  k[diag]=0             (dist normalized by sqrt(D)=8,
                                                         TEMP=0.05 -> exp(-2.5*sqrt(d2)))
  row[g]   = sum_t k[t,g];  col[t] = sum_g k[t,g]
  nk       = k * min(rsqrt(row)*rsqrt(col), 1e6)        (== k / sqrt(max(row*col,1e-12)))
  s_gen[g] = sum_{t<G} nk ; s_pos[g] = sum_{t>=G} nk
  M_pos    = nk[G:].T @ pos ; M_neg = nk[:G].T @ gen    (transposed layout)
  V        = s_gen*M_pos - s_pos*M_neg
  loss    += sum(V^2);  drift += sum_g ||V[g]||

One class per NeuronCore (8 classes / 8 cores), no collectives; host sums
the 8 scalar pairs.

Implementation notes:
  - d2 via Gram matmuls in float32r (fp32 data, 1 cycle/row on PE for
    512-wide outputs vs 4 for plain fp32).
  - 2-pass ACT chain instead of the 3-pass ln/exp/exp: S = sqrt(6.25*d2
    + eps) then k = exp(-S) with column-sum accumulation.  sqrt and exp
    live in different activation-table sets, so chunks are processed in
    groups of GK between table switches (1283ns per load), with S parked
    in a [128, GK, G] f32 scratch.
  - Row sums: pos chunks via ones-matmuls; the gen-gen block is
    symmetric so its row sums equal colacc (transposed into the same
    PSUM accumulators by a colacc x identity matmul).
  - Phase 2 re-reads the persistent bf16 KT: W = min(bcast(rsqrt(row)) *
    rsqrt(col)_t, 1e6) (DVE 4x mode), nk chunk = KT*W (DVE 2x), side
    matmuls psum[side] += aug(side).T @ nk with aug = [x, 1] so row 64
    holds s_gen/s_pos.
  - Diagonal masking: BIG*I matmul accumulated onto the d2 PSUM makes
    k_diag = exp(-sqrt(6.25*BIG)) underflow to 0 in bf16.
"""

import sys

for _p in ("/opt/trn_rl_repo", "/root/.axon_site/_ro/trn_rl_repo"):
    if _p not in sys.path:
        sys.path.insert(0, _p)

import ml_dtypes
import numpy as np

C = 8
BIG = 4000.0  # diag d2 offset: exp(-sqrt(6.25*4000)) = exp(-158) -> 0 in bf16
EPS = 0.01  # sqrt(6.25*d2 + EPS); guards d2 ~ -1e-4 roundoff on the diagonal
GK = 8  # chunks per activation-table group

_CACHE = {}


def _patch_act_tables():
    """Restrict ACT table sets so the load inserter never thrashes: Sqrt
    lives only in sqrt_and_others, Ln/Exp only in natural_log_exp_and_others
    (the default sets split ln and exp across natural_log / exp_and_others,
    which makes every Ln->Exp pair reload a table).  Set IDs stay aligned
    with the compiler's act_info.json because only membership is filtered."""
    import functools

    import concourse.bacc as bacc
    import concourse.hw_specs as hw_specs

    if getattr(hw_specs.get_activation_tables, "_drift_patched", False):
        return
    orig = hw_specs.get_activation_tables

    @functools.cache
    def patched(module_arch):
        keep = ("natural_log_exp_and_others", "sqrt_and_others")
        return {
            name: (funcs if name in keep else set())
            for name, funcs in orig(module_arch).items()
        }

    patched._drift_patched = True
    hw_specs.get_activation_tables = patched
    bacc.get_activation_tables = patched


def _build(G, P):
    import concourse.bacc as bacc
    import concourse.tile as tile
    from concourse import mybir

    _patch_act_tables()

    f32 = mybir.dt.float32
    f32r = mybir.dt.float32r
    f16 = mybir.dt.float16
    bf16 = mybir.dt.bfloat16
    AF = mybir.ActivationFunctionType
    OP = mybir.AluOpType

    T = G + P
    NT = T // 128  # t-chunks (partition dim of KT)
    NG = G // 128  # gen-block t-chunks
    RW = 512  # matmul slice width (psum bank limit: 512 fp32)
    GR = G // RW
    HW = G // 2  # d2 half width (2 psum banks per half)
    assert T % 128 == 0 and G % 512 == 0 and P % 128 == 0 and NT % GK == 0

    nc = bacc.Bacc("TRN2", target_bir_lowering=False, debug=False, num_devices=8)

    ta = nc.dram_tensor("ta", [66, T], f32r, kind="ExternalInput")
    gb = nc.dram_tensor("gb", [66, G], f32r, kind="ExternalInput")
    posa = nc.dram_tensor("posa", [128, P // 128, 65], bf16, kind="ExternalInput")
    gena = nc.dram_tensor("gena", [128, G // 128, 65], bf16, kind="ExternalInput")
    ident = nc.dram_tensor("ident", [128, 128], bf16, kind="ExternalInput")
    identf = nc.dram_tensor("identf", [128, 128], f32, kind="ExternalInput")
    bigi = nc.dram_tensor("bigi", [128, 128], bf16, kind="ExternalInput")
    ones_r = nc.dram_tensor("ones_r", [128, 1], bf16, kind="ExternalInput")
    ones_b = nc.dram_tensor("ones_b", [1, 128], f32, kind="ExternalInput")
    ones_v = nc.dram_tensor("ones_v", [64, 1], f32, kind="ExternalInput")
    outh = nc.dram_tensor("out", [1, 2], f32, kind="ExternalOutput")

    from contextlib import ExitStack

    with tile.TileContext(nc) as tc, ExitStack() as ctx:
        kpool = ctx.enter_context(tc.tile_pool(name="kpool", bufs=1))
        bigp = ctx.enter_context(tc.tile_pool(name="bigp", bufs=1))
        singles = ctx.enter_context(tc.tile_pool(name="singles", bufs=1))
        smalls = ctx.enter_context(tc.tile_pool(name="smalls", bufs=1))

        # persistent kernel matrix, [128, NT, G] bf16 (t-chunk major)
        KT = kpool.tile([128, NT, G], bf16)

        POSAs = singles.tile([128, P // 128, 65], bf16)
        GENAs = singles.tile([128, G // 128, 65], bf16)
        IDENTs = singles.tile([128, 128], bf16)
        IDENTFs = singles.tile([128, 128], f32)
        BIGIs = singles.tile([128, 128], bf16)
        ONESRs = singles.tile([128, 1], bf16)
        ONESBs = singles.tile([1, 128], f32)
        ONESVs = singles.tile([64, 1], f32)

        epsb = smalls.tile([128, 1], f32)
        qeps = smalls.tile([1, 1], f32)
        colacc = smalls.tile([128, NT], f32)
        lnc = smalls.tile([128, NT], f32)
        bvec = smalls.tile([128, NT], f32)
        lsums = smalls.tile([1, GR], f32)
        dsums = smalls.tile([1, GR], f32)

        rowS = bigp.tile([1, G], f32, tag="rowS")
        BCA = bigp.tile([128, G], bf16, tag="bca")

        order = list(range(NG, NT)) + list(range(NG))  # pos chunks first

        # phase-1-only tensors live in their own pool, freed before phase 2
        with tc.tile_pool(name="ph1", bufs=1) as p1:
            TAs = p1.tile([66, T], f32r, tag="ta")
            GBs = p1.tile([66, G], f32r, tag="gb")
            S = p1.tile([128, GK, G], f16, tag="s")  # sqrt scratch (fp16: ~5e-4 rel on S)

            # DMA order: the chunk loop starts at the pos block (i=NG), so
            # its TA quarter + GB must land first; late-needed tensors ride
            # the queue tails.  ACT queue stays DMA-free (first instruction
            # must be the sqrt table load).
            h, q3 = T // 2, 3 * T // 4
            nc.sync.dma_start(out=TAs[:, h : h + 256], in_=ta[:, h : h + 256])
            nc.gpsimd.dma_start(out=GBs[:, :512], in_=gb[:, :512])
            nc.sync.dma_start(out=TAs[:, h + 256 : q3], in_=ta[:, h + 256 : q3])
            nc.sync.dma_start(out=TAs[:, q3:], in_=ta[:, q3:])
            nc.gpsimd.dma_start(out=GBs[:, 512 : G // 2], in_=gb[:, 512 : G // 2])
            nc.gpsimd.dma_start(out=GBs[:, G // 2 :], in_=gb[:, G // 2 :])
            nc.gpsimd.dma_start(out=ONESRs[:], in_=ones_r[:, :])
            nc.gpsimd.dma_start(out=POSAs[:], in_=posa[:, :, :])
            nc.gpsimd.dma_start(out=GENAs[:], in_=gena[:, :, :])
            nc.sync.dma_start(out=IDENTs[:], in_=ident[:, :])
            nc.sync.dma_start(out=IDENTFs[:], in_=identf[:, :])
            nc.sync.dma_start(out=BIGIs[:], in_=bigi[:, :])
            nc.sync.dma_start(out=ONESBs[:], in_=ones_b[:, :])
            nc.sync.dma_start(out=ONESVs[:], in_=ones_v[:, :])
            nc.sync.dma_start(out=TAs[:, :h], in_=ta[:, :h])

            nc.vector.memset(epsb[:], EPS)
            nc.vector.memset(qeps[:], 1.0e-35)

            groups = [order[g * GK : (g + 1) * GK] for g in range(NT // GK)]

            # ---- Phase 1: build KT (sqrt/exp table groups), col + row sums
            with (
                tc.tile_pool(name="d2p", bufs=1, space="PSUM") as dp,
                tc.tile_pool(name="rap", bufs=1, space="PSUM") as rp,
            ):
                rowaccs = [
                    rp.tile([1, RW], f32, tag=f"ra{j}", name=f"rowacc{j}")
                    for j in range(GR)
                ]
                for grp in groups:
                    # sqrt phase: d2 matmuls ping-pong 2 psum half-tiles
                    for slot, i in enumerate(grp):
                        jd = i // (RW // 128) if i < NG else -1
                        for hh in range(2):
                            d2 = dp.tile(
                                [128, HW], f32, tag=f"d2{hh}", name=f"d2{hh}"
                            )
                            for jj in range(HW // RW):
                                j = hh * (HW // RW) + jj
                                nc.tensor.matmul(
                                    d2[:, jj * RW : (jj + 1) * RW],
                                    TAs[:, i * 128 : (i + 1) * 128],
                                    GBs[:, j * RW : (j + 1) * RW],
                                    start=True,
                                    stop=(j != jd),
                                    skip_group_check=True,
                                )
                                if j == jd:
                                    c0 = i * 128 - hh * HW
                                    nc.tensor.matmul(
                                        d2[:, c0 : c0 + 128],
                                        IDENTs[:],
                                        BIGIs[:],
                                        start=False,
                                        stop=True,
                                        skip_group_check=True,
                                    )
                            nc.scalar.activation(
                                S[:, slot, hh * HW : (hh + 1) * HW],
                                d2[:],
                                AF.Sqrt,
                                bias=epsb[:],
                                scale=6.25,
                            )
                    # exp phase: k chunks -> KT + col sums; row-sum matmuls
                    for slot, i in enumerate(grp):
                        nc.scalar.activation(
                            KT[:, i, :], S[:, slot, :], AF.Exp, scale=-1.0,
                            accum_out=colacc[:, i : i + 1],
                        )
                        if i < NG:
                            # gen-gen block symmetric: its row sums ==
                            # colacc; transpose into the rowacc banks
                            j, m = divmod(i * 128, RW)
                            nc.tensor.matmul(
                                rowaccs[j][:, m : m + 128],
                                colacc[:, i : i + 1],
                                IDENTFs[:],
                                start=False,
                                stop=(m + 128 == RW),
                                skip_group_check=True,
                            )
                            if m + 128 == RW:
                                # slice j's row sums are complete: fold in
                                # a=rsqrt(row) and broadcast to BCA now,
                                # overlapped under the remaining chunks
                                js = slice(j * RW, (j + 1) * RW)
                                nc.scalar.activation(
                                    rowS[:, js], rowaccs[j][:], AF.Ln
                                )
                                nc.scalar.activation(
                                    rowS[:, js], rowS[:, js], AF.Exp,
                                    scale=-0.5,
                                )
                                pb = rp.tile(
                                    [128, RW], f32, tag=f"ra{j}",
                                    name=f"bc{j}",
                                )
                                nc.tensor.matmul(
                                    pb[:], ONESBs[:], rowS[:, js],
                                    start=True, stop=True,
                                    skip_group_check=True,
                                )
                                nc.vector.tensor_copy(BCA[:, js], pb[:])
                        else:
                            for j in range(GR):
                                nc.tensor.matmul(
                                    rowaccs[j][:],
                                    ONESRs[:],
                                    KT[:, i, j * RW : (j + 1) * RW],
                                    start=(i == NG),
                                    stop=False,
                                    skip_group_check=True,
                                )
            # b = rsqrt(col) per t (per-partition, [128, NT])
            nc.scalar.activation(lnc[:], colacc[:], AF.Ln)
            nc.scalar.activation(bvec[:], lnc[:], AF.Exp, scale=-0.5)

        # ---- Phase 2: nk chunks = KT * min(a*b, 1e6), side matmuls
        # (ph2 pool opens after ph1 closed, reusing its SBUF region)
        ph2 = ctx.enter_context(tc.tile_pool(name="ph2", bufs=1))
        PNo = ph2.tile([65, G], f32, tag="pn")
        PPo = ph2.tile([65, G], f32, tag="pp")
        with (
            tc.tile_pool(name="wp", bufs=4) as wp,
            tc.tile_pool(name="nkp", bufs=4) as nkp,
            tc.tile_pool(name="sp2", bufs=1, space="PSUM") as p2,
        ):
            psums = None
            for i in order:
                if i in (NG, 0):  # entering pos / gen half: (re)use banks
                    psums = [
                        p2.tile([65, RW], f32, tag=f"sp{j}", name=f"sp{j}_{i}")
                        for j in range(GR)
                    ]
                W = wp.tile([128, G], bf16, tag="w")
                nc.vector.tensor_scalar(
                    W[:], BCA[:], bvec[:, i : i + 1], 1.0e6,
                    op0=OP.mult, op1=OP.min,
                )
                NK = nkp.tile([128, G], bf16, tag="nk")
                # DVE is the phase-2 bottleneck; GpSimd runs tensor_tensor
                # at ~1.7us vs DVE's ~0.9us, so give it 5 of every 8 chunks
                eng = nc.gpsimd if i % 2 == 0 else nc.vector
                eng.tensor_mul(NK[:], KT[:, i, :], W[:])
                side = 0 if i < NG else 1
                lhs = GENAs[:, i, :] if side == 0 else POSAs[:, i - NG, :]
                first = i in (0, NG)
                last = i in (NG - 1, NT - 1)
                for j in range(GR):
                    nc.tensor.matmul(
                        psums[j][:],
                        lhs,
                        NK[:, j * RW : (j + 1) * RW],
                        start=first,
                        stop=last,
                        skip_group_check=True,
                    )
                if i == NT - 1:  # pos side complete -> drain (ACT is idle)
                    for j in range(GR):
                        nc.scalar.activation(
                            PPo[:, j * RW : (j + 1) * RW], psums[j][:], AF.Copy
                        )
                if i == NG - 1:  # gen side complete -> drain
                    for j in range(GR):
                        nc.scalar.activation(
                            PNo[:, j * RW : (j + 1) * RW], psums[j][:], AF.Copy
                        )

        # ---- tail: V.T = bcast(s_gen)*M_pos.T - bcast(s_pos)*M_neg.T
        # rows 0:64 of PNo/PPo = M_neg.T / M_pos.T ; row 64 = s_gen / s_pos
        qS = bigp.tile([1, G], f32, tag="rowS")  # reuse the rowS region
        with (
            tc.tile_pool(name="vtp", bufs=1) as vtp,
            tc.tile_pool(name="bc2", bufs=2, space="PSUM") as bp2,
            tc.tile_pool(name="qp", bufs=2, space="PSUM") as qp,
        ):
            for j in range(GR):
                js = slice(j * RW, (j + 1) * RW)
                sgr = vtp.tile([1, RW], f32, tag="sgr")
                spr = vtp.tile([1, RW], f32, tag="spr")
                nc.vector.tensor_copy(sgr[:], PNo[64:65, js])
                nc.vector.tensor_copy(spr[:], PPo[64:65, js])
                bg = bp2.tile([64, RW], f32, tag="bg")
                bpp = bp2.tile([64, RW], f32, tag="bp")
                nc.tensor.matmul(
                    bg[:], ONESBs[:, 0:64], sgr[:],
                    start=True, stop=True, skip_group_check=True,
                )
                nc.tensor.matmul(
                    bpp[:], ONESBs[:, 0:64], spr[:],
                    start=True, stop=True, skip_group_check=True,
                )
                vt1 = vtp.tile([64, RW], f32, tag="vt1", bufs=2)
                vt2 = vtp.tile([64, RW], f32, tag="vt2", bufs=2)
                nc.vector.tensor_mul(vt1[:], PPo[0:64, js], bg[:])
                nc.vector.tensor_mul(vt2[:], PNo[0:64, js], bpp[:])
                nc.vector.tensor_sub(vt1[:], vt1[:], vt2[:])
                nc.vector.tensor_mul(vt2[:], vt1[:], vt1[:])  # V^2
                qt = qp.tile([1, RW], f32, tag="q")
                nc.tensor.matmul(
                    qt[:], ONESVs[:], vt2[:], start=True, stop=True,
                    skip_group_check=True,
                )
                # loss partial = sum(q) via copy-with-accum; drift partial =
                # sum(sqrt(q)) via ln/exp with accum
                nc.vector.tensor_scalar(
                    qS[:, js], qt[:], 1.0, 0.0, op0=OP.mult, op1=OP.add,
                    accum_out=lsums[:, j : j + 1],
                )
                nc.scalar.activation(qS[:, js], qS[:, js], AF.Ln, bias=qeps[:])
                nc.scalar.activation(
                    qS[:, js], qS[:, js], AF.Exp, scale=0.5,
                    accum_out=dsums[:, j : j + 1],
                )

        outS = smalls.tile([1, 2], f32)
        nc.vector.tensor_reduce(
            outS[:, 0:1], lsums[:], axis=mybir.AxisListType.X, op=OP.add
        )
        nc.vector.tensor_reduce(
            outS[:, 1:2], dsums[:], axis=mybir.AxisListType.X, op=OP.add
        )
        nc.sync.dma_start(out=outh[:, :], in_=outS[:])

    nc.compile()
    return nc


def _prep_class(gen_c, pos_c):
    """Host-side input prep for one class -> dict of named arrays."""
    gen_c = np.ascontiguousarray(gen_c, dtype=np.float32)
    pos_c = np.ascontiguousarray(pos_c, dtype=np.float32)
    G, D = gen_c.shape
    targets = np.concatenate([gen_c, pos_c], axis=0)
    T = targets.shape[0]

    ta = np.empty((66, T), np.float32)
    ta[0:64] = -2.0 * targets.T
    ta[64] = (targets * targets).sum(axis=1)
    ta[65] = 1.0

    gbm = np.empty((66, G), np.float32)
    gbm[0:64] = gen_c.T
    gbm[64] = 1.0
    gbm[65] = (gen_c * gen_c).sum(axis=1)

    def aug(x):
        n = x.shape[0]
        a = np.empty((n, 65), np.float32)
        a[:, 0:64] = x
        a[:, 64] = 1.0
        return (
            a.astype(ml_dtypes.bfloat16)
            .reshape(n // 128, 128, 65)
            .transpose(1, 0, 2)
            .copy()
        )

    bf = ml_dtypes.bfloat16
    return {
        "ta": ta,
        "gb": gbm,
        "posa": aug(pos_c),
        "gena": aug(gen_c),
        "ident": np.eye(128, dtype=bf),
        "identf": np.eye(128, dtype=np.float32),
        "bigi": (BIG * np.eye(128)).astype(bf),
        "ones_r": np.ones((128, 1), bf),
        "ones_b": np.ones((1, 128), np.float32),
        "ones_v": np.ones((64, 1), np.float32),
    }


def kernel(generated, labels_gen, positive, labels_pos):
    from concourse.bass_utils import run_bass_kernel_spmd

    generated = np.asarray(generated, dtype=np.float32)
    positive = np.asarray(positive, dtype=np.float32)
    N, D = generated.shape
    Np = positive.shape[0]
    G, P = N // C, Np // C
    assert D == 64

    key = (G, P)
    if key not in _CACHE:
        _CACHE[key] = _build(G, P)
    nc = _CACHE[key]

    in_maps = [
        _prep_class(
            generated[c * G : (c + 1) * G], positive[c * P : (c + 1) * P]
        )
        for c in range(C)
    ]
    res = run_bass_kernel_spmd(nc, in_maps, core_ids=list(range(C)))
    sums = np.stack([res.results[i]["out"][0] for i in range(C)])  # [C, 2]
    loss = sums[:, 0].sum() / (N * D)
    dn = sums[:, 1].sum() / N
    return np.float32(loss), np.float32(dn)


if __name__ == "__main__":
    rng = np.random.default_rng(0)
    N = 16384
    gen = rng.standard_normal((N, 64), dtype=np.float32)
    pos = rng.standard_normal((N, 64), dtype=np.float32)
    lg = np.repeat(np.arange(C), N // C).astype(np.int32)
    print(kernel(gen, lg, pos, lg))
